# revision 1
# baseline (speedup 1.0000x reference)
"""Trainium2 Bass kernel for a 6-layer BERT encoder (nn_BertEncoder).

Strategy: data-parallel over batch across 8 NeuronCores (16 batches -> 2 per
core), no collectives.  Inside each core the encoder runs with feature-major
("transposed") activations so every projection uses the weight as the PE
stationary operand with zero transposes; attention computes transposed scores
so the softmax denominator falls out of an appended ones-column of V; layer
norm uses ones-vector matmuls for the partition reductions.  Matmuls run in
bf16 (f32 accumulation), layernorm/residual paths stay f32.
"""

import sys

sys.path.insert(0, "/opt/trn_rl_repo")

from contextlib import ExitStack

import numpy as np
import ml_dtypes

import concourse.bass as bass
import concourse.mybir as mybir
import concourse.tile as tile
from concourse.vector_clock import ScopedClock, VectorClock

F32 = mybir.dt.float32
F32R = mybir.dt.float32r
BF16 = mybir.dt.bfloat16
AF = mybir.ActivationFunctionType
ALU = mybir.AluOpType

H = 768
I = 3072
NH = 12
DH = 64
KH = H // 128  # 6 feature slabs
KI = I // 128  # 24 intermediate chunks
EPS = 1e-12


class SplitDrainTileContext(tile.TileContext):
    """TileContext whose kernel-tail drain splits its semaphore waits across
    several SP nops -- this walrus build rejects >1 sync wait on a TPB_CTRL
    (Drain/NoOp) instruction."""

    def _drain_and_barrier(self, tick_clock, wait_clock):
        gc = tick_clock.global_clock
        nprocs = len(gc)
        procs = [p for p in range(nprocs) if gc[p] > 0]
        covered = [0] * nprocs
        for p in procs:
            req = [0] * nprocs
            req[p] = gc[p]
            nop_inst = self.nc.sync.nop(nofuse=True)
            wait_clock.add_sem_waits(
                nop_inst.ins,
                ScopedClock({None: VectorClock(req)}),
                ScopedClock({None: VectorClock(list(covered))}),
            )
            covered[p] = gc[p]
        drain_inst = self.nc.sync.drain()
        wait_clock.add_sem_waits(
            drain_inst.ins,
            ScopedClock({None: gc.copy()}),
            ScopedClock({None: VectorClock(list(covered))}),
        )
        self.nc.all_engine_barrier()
        assert self.sems is not None
        popped = self.nc._tile_sem_poison_stack.pop()
        assert popped is self._sem_poison
        self.nc.clear_and_free_semaphores(list(self.sems.allocated().values()))
        self.nc.all_engine_barrier()




def split_multi_waits(nc):
    """Walrus in this container accepts at most ONE sync wait per
    instruction.  Split every instruction carrying N>1 waits into N-1
    same-engine NOPs (each holding one wait) followed by the original
    instruction with the final wait."""
    f = nc.m.functions[0]
    n_split = 0
    for bb in f.blocks:
        insts = list(bb.instructions)
        out = []
        changed = False
        for inst in insts:
            si = inst.sync_info
            if si is not None and len(si.on_wait) > 1:
                waits = list(si.on_wait)
                for j, w in enumerate(waits[:-1]):
                    nop = mybir.InstNoOp(
                        name=f"{inst.name}_sw{j}",
                        engine=inst.engine,
                        sync_info=mybir.SyncInfo(on_wait=[w], on_update=[]),
                        bass_nofuse=True,
                    )
                    out.append(nop)
                inst.sync_info = mybir.SyncInfo(
                    on_wait=[waits[-1]], on_update=list(si.on_update)
                )
                n_split += 1
                changed = True
            out.append(inst)
        if changed:
            bb.instructions = out
    # verify the mutation took effect (blocks could have been copies)
    for bb in nc.m.functions[0].blocks:
        for inst in bb.instructions:
            si = inst.sync_info
            assert si is None or len(si.on_wait) <= 1, (
                f"multi-wait survived on {inst.name}"
            )
    return n_split


def build_nc(n_layers: int, b_local: int, s: int, num_devices: int = 8, split_waits: bool = True):
    """Build the per-core Bass module.  Per-core DRAM parameters:

      hT    [128, KH, T] f32      transposed hidden states
      wq/wk/wv/wo [L, KH, 128, H] bf16   weight k-slabs (in-major)
      wi    [L, KH, 128, I] bf16
      wo2   [L, KI, 128, H] bf16
      cq/ck/co/cg1/cb1/co2/cg2/cb2 [L, 128, KH] f32  per-partition bias cols
      ci    [L, 128, KI] f32
      bvrow [L, 1, H] f32
      outT  [128, KH, T] f32
    """
    T = b_local * s
    NT = T // 128  # token chunks
    NS = T // 512  # 512-wide token column blocks
    nc = bass.Bass(
        "TRN2", target_bir_lowering=False, debug=False, num_devices=num_devices
    )

    hT = nc.dram_tensor("hT", [128, KH, T], F32, kind="ExternalInput").ap()
    L = n_layers
    wq = nc.dram_tensor("wq", [L, KH, 128, H], BF16, kind="ExternalInput").ap()
    wk = nc.dram_tensor("wk", [L, KH, 128, H], BF16, kind="ExternalInput").ap()
    wv = nc.dram_tensor("wv", [L, KH, 128, H], BF16, kind="ExternalInput").ap()
    wo = nc.dram_tensor("wo", [L, KH, 128, H], BF16, kind="ExternalInput").ap()
    wi = nc.dram_tensor("wi", [L, KH, 128, I], BF16, kind="ExternalInput").ap()
    wo2 = nc.dram_tensor("wo2", [L, KI, 128, H], BF16, kind="ExternalInput").ap()
    # all per-partition bias/gain columns packed: 8 groups of KH + KI for ci
    cols = nc.dram_tensor(
        "cols", [L, 128, 8 * KH + KI], F32, kind="ExternalInput"
    ).ap()
    bvrow = nc.dram_tensor("bvrow", [L, 1, H], F32, kind="ExternalInput").ap()
    outT = nc.dram_tensor("outT", [128, KH, T], F32, kind="ExternalOutput").ap()

    with SplitDrainTileContext(nc) as tc:
        build_body(
            tc,
            n_layers,
            b_local,
            s,
            hT,
            (wq, wk, wv, wo, wi, wo2),
            (cols, bvrow),
            outT,
        )
    if split_waits:
        split_multi_waits(nc)
    return nc


def build_body(tc, n_layers, b_local, s, hT, weights, biases, outT):
    nc = tc.nc
    T = b_local * s
    NT = T // 128
    NS = T // 512
    SC = s // 128  # score row chunks per (batch, head) = 4
    wq, wk, wv, wo, wi, wo2 = weights
    cols, bvrow = biases

    VW = DH + 1  # v_aug per-head width (64 + ones column)

    # ---- persistent pools (kept open for the whole kernel) ----------------
    ctx_stack = ExitStack()
    ec = ctx_stack.enter_context
    big = ec(tc.tile_pool(name="big_f32", bufs=2))  # x1 / x2+out
    actb = ec(tc.tile_pool(name="act_bf16", bufs=2))  # h/ctx/xsq/attn
    qkp = ec(tc.tile_pool(name="qk", bufs=2))
    vap = ec(tc.tile_pool(name="vaug", bufs=1))
    expp = ec(tc.tile_pool(name="exp", bufs=4))
    rbp = ec(tc.tile_pool(name="rbcast", bufs=2))
    lnb = ec(tc.tile_pool(name="lnbcast", bufs=2))
    rows = ec(tc.tile_pool(name="rows", bufs=2))
    rrp = ec(tc.tile_pool(name="rrp", bufs=2))
    wpj = ec(tc.tile_pool(name="wproj", bufs=4))
    wib = ec(tc.tile_pool(name="wiblk", bufs=2))
    w2p = ec(tc.tile_pool(name="wo2", bufs=2))
    bcol = ec(tc.tile_pool(name="bcols", bufs=2))
    gel = ec(tc.tile_pool(name="gelu", bufs=3))
    ones_pool = ec(tc.tile_pool(name="ones", bufs=1))

    ones_f32 = ones_pool.tile([128, 1], F32)
    nc.vector.memset(ones_f32, 1.0)
    ones_bf16 = ones_pool.tile([128, 1], BF16)
    nc.vector.memset(ones_bf16, 1.0)
    eps_t = ones_pool.tile([128, 1], F32)
    nc.vector.memset(eps_t, EPS)
    ones_f32row = ones_pool.tile([1, 128], F32)
    nc.vector.memset(ones_f32row, 1.0)
    ones_brow = ones_pool.tile([1, 128], BF16)
    nc.vector.memset(ones_brow, 1.0)

    # ---- load initial hidden state ---------------------------------------
    h = big.tile([128, KH, T], F32, tag="bigbuf")
    nc.sync.dma_start(out=h[:], in_=hT)
    h_bf = actb.tile([128, KH, T], BF16, tag="actb")
    nc.scalar.activation(out=h_bf[:], in_=h[:], func=AF.Copy)

    def layernorm(x, g_col, b_col, out_bf, psum_ln, psum_bc, last=False):
        """LN over features (partitions across KH slabs) of x [128,KH,T] f32.
        Applies in place (x <- LN(x)*g+b); writes bf16 copy to out_bf unless
        last.  Stats: bf16 rounded copy + square, summed over partitions with
        ones-vector matmuls; all row math stays on partition 0."""
        xb = actb.tile([128, KH, T], BF16, tag="actb")
        nc.scalar.activation(out=xb[:], in_=x[:], func=AF.Copy)
        xsq = actb.tile([128, KH, T], BF16, tag="actb")
        nc.vector.tensor_tensor(xsq[:], xb[:], xb[:], op=ALU.mult)
        srow = rows.tile([1, 3 * T], F32, tag="srow", bufs=1)
        mean_row = srow[:, 0:T]
        msq_row = srow[:, T : 2 * T]
        var_row = srow[:, 2 * T : 3 * T]
        for n in range(NS):
            tsl = bass.ts(n, 512)
            ps_sum = psum_ln.tile([1, 512], F32, tag="lnp")
            for k in range(KH):
                nc.tensor.matmul(
                    ps_sum,
                    lhsT=ones_bf16,
                    rhs=xb[:, k, tsl],
                    start=(k == 0),
                    stop=(k == KH - 1),
                )
            ps_sq = psum_ln.tile([1, 512], F32, tag="lnp")
            for k in range(KH):
                nc.tensor.matmul(
                    ps_sq,
                    lhsT=ones_bf16,
                    rhs=xsq[:, k, tsl],
                    start=(k == 0),
                    stop=(k == KH - 1),
                )
            nc.scalar.activation(
                out=mean_row[:, tsl], in_=ps_sum, func=AF.Copy, scale=1.0 / H
            )
            nc.scalar.activation(
                out=msq_row[:, tsl], in_=ps_sq, func=AF.Copy, scale=1.0 / H
            )
        nc.vector.tensor_tensor(var_row, mean_row, mean_row, op=ALU.mult)
        nc.vector.tensor_tensor(var_row, msq_row, var_row, op=ALU.subtract)
        nc.scalar.activation(out=var_row, in_=var_row, func=AF.Sqrt, bias=eps_t[0:1, :])
        nc.vector.reciprocal(var_row, var_row)
        rstd_row = var_row
        mean_b = lnb.tile([128, T], F32, tag="lnb")
        rstd_b = lnb.tile([128, T], F32, tag="lnb")
        for n in range(NS):
            tsl = bass.ts(n, 512)
            for row, dst in ((mean_row, mean_b), (rstd_row, rstd_b)):
                ps_bc = psum_bc.tile([128, 512], F32, tag="lnbc")
                nc.tensor.matmul(
                    ps_bc, lhsT=ones_f32row, rhs=row[:, tsl], start=True, stop=True
                )
                nc.scalar.activation(out=dst[:, tsl], in_=ps_bc, func=AF.Copy)
        for k in range(KH):
            nc.vector.tensor_tensor(x[:, k, :], x[:, k, :], mean_b, op=ALU.subtract)
            nc.vector.tensor_tensor(x[:, k, :], x[:, k, :], rstd_b, op=ALU.mult)
            # bf16 copy (with per-partition gain/bias) on ACT, f32 on DVE
            if not last:
                nc.scalar.activation(
                    out=out_bf[:, k, :],
                    in_=x[:, k, :],
                    func=AF.Identity,
                    scale=g_col[:, k : k + 1],
                    bias=b_col[:, k : k + 1],
                )
            nc.vector.tensor_scalar(
                out=x[:, k, :],
                in0=x[:, k, :],
                scalar1=g_col[:, k : k + 1],
                scalar2=b_col[:, k : k + 1],
                op0=ALU.mult,
                op1=ALU.add,
            )

    for l in range(n_layers):
        # ---- per-layer constants (single DMA for all bias columns) -------
        cols_t = bcol.tile([128, 8 * KH + KI], F32, tag="cols")
        nc.sync.dma_start(out=cols_t[:], in_=cols[l])
        cq_t = cols_t[:, 0 * KH : 1 * KH]
        ck_t = cols_t[:, 1 * KH : 2 * KH]
        co_t = cols_t[:, 2 * KH : 3 * KH]
        cg1_t = cols_t[:, 3 * KH : 4 * KH]
        cb1_t = cols_t[:, 4 * KH : 5 * KH]
        co2_t = cols_t[:, 5 * KH : 6 * KH]
        cg2_t = cols_t[:, 6 * KH : 7 * KH]
        cb2_t = cols_t[:, 7 * KH : 8 * KH]
        ci_t = cols_t[:, 8 * KH : 8 * KH + KI]
        bvbuf = rows.tile([1, max(T, H)], F32, tag="bvrow", bufs=1)
        bv_r = bvbuf[0:1, 0:H]
        nc.sync.dma_start(out=bv_r, in_=bvrow[l])
        bv_b = bcol.tile([128, H], F32, tag="bvb", bufs=1)

        wq_t = wpj.tile([128, KH, H], BF16, tag="wpj", name=f"wq_{l}")
        nc.sync.dma_start(out=wq_t[:], in_=wq[l].rearrange("k p h -> p k h"))
        wk_t = wpj.tile([128, KH, H], BF16, tag="wpj", name=f"wk_{l}")
        nc.sync.dma_start(out=wk_t[:], in_=wk[l].rearrange("k p h -> p k h"))
        wv_t = wpj.tile([128, KH, H], BF16, tag="wpj", name=f"wv_{l}")
        nc.sync.dma_start(out=wv_t[:], in_=wv[l].rearrange("k p h -> p k h"))
        wo_t = wpj.tile([128, KH, H], BF16, tag="wpj", name=f"wo_{l}")
        nc.sync.dma_start(out=wo_t[:], in_=wo[l].rearrange("k p h -> p k h"))
        wq_s = [wq_t[:, k, :] for k in range(KH)]
        wk_s = [wk_t[:, k, :] for k in range(KH)]
        wv_s = [wv_t[:, k, :] for k in range(KH)]
        wo_s = [wo_t[:, k, :] for k in range(KH)]

        # ---- phase A: QKV projections ------------------------------------
        qT = qkp.tile([128, KH, T], BF16, tag="qk")
        kT = qkp.tile([128, KH, T], BF16, tag="qk")
        v_aug = vap.tile([128, NT, NH * VW], BF16, tag="vaug")
        with tc.tile_pool(name="psA", bufs=6, space="PSUM") as psA:
            for f0, fw in ((0, 512), (512, 256)):
                ps_bv = psA.tile([128, 512], F32, tag="psA")
                nc.tensor.matmul(
                    ps_bv[:, 0:fw],
                    lhsT=ones_f32row,
                    rhs=bv_r[:, f0 : f0 + fw],
                    start=True,
                    stop=True,
                )
                nc.vector.tensor_copy(out=bv_b[:, f0 : f0 + fw], in_=ps_bv[:, 0:fw])
            for m in range(KH):
                for n in range(NS):
                    tsl = bass.ts(n, 512)
                    msl = bass.ts(m, 128)
                    ps = psA.tile([128, 512], F32, tag="psA")
                    for k in range(KH):
                        nc.tensor.matmul(
                            ps,
                            lhsT=wq_s[k][:, msl],
                            rhs=h_bf[:, k, tsl],
                            start=(k == 0),
                            stop=(k == KH - 1),
                        )
                    nc.scalar.activation(
                        out=qT[:, m, tsl],
                        in_=ps,
                        func=AF.Identity,
                        bias=cq_t[:, m : m + 1],
                    )
                    ps2 = psA.tile([128, 512], F32, tag="psA")
                    for k in range(KH):
                        nc.tensor.matmul(
                            ps2,
                            lhsT=wk_s[k][:, msl],
                            rhs=h_bf[:, k, tsl],
                            start=(k == 0),
                            stop=(k == KH - 1),
                        )
                    nc.scalar.activation(
                        out=kT[:, m, tsl],
                        in_=ps2,
                        func=AF.Identity,
                        bias=ck_t[:, m : m + 1],
                    )
            # V in natural layout (tokens on partitions), heads interleaved
            # with a ones column:  v_aug[:, mt, h*VW : h*VW+64] = v tokens x dh
            for mt in range(NT):
                tsl = bass.ts(mt, 128)
                nc.vector.memset(
                    v_aug[:, mt, :].rearrange("p (h d) -> p h d", d=VW)[:, :, DH:VW],
                    1.0,
                )
                for half, (f0, fw) in enumerate([(0, 512), (512, 256)]):
                    ps = psA.tile([128, 512], F32, tag="psA")
                    for k in range(KH):
                        nc.tensor.matmul(
                            ps[:, 0:fw],
                            lhsT=h_bf[:, k, tsl],
                            rhs=wv_s[k][:, f0 : f0 + fw],
                            start=(k == 0),
                            stop=(k == KH - 1),
                        )
                    nh0 = f0 // DH
                    nh = fw // DH
                    out_ap = v_aug[:, mt, nh0 * VW : (nh0 + nh) * VW].rearrange(
                        "p (h d) -> p h d", d=VW
                    )[:, :, 0:DH]
                    nc.vector.tensor_tensor(
                        out_ap,
                        ps[:, 0:fw].rearrange("p (h d) -> p h d", d=DH),
                        bv_b[:, f0 : f0 + fw].rearrange("p (h d) -> p h d", d=DH),
                        op=ALU.add,
                    )

        # ---- phase B: attention ------------------------------------------
        ctx = actb.tile([128, KH, T], BF16, tag="actb")
        with (
            tc.tile_pool(name="psS", bufs=4, space="PSUM") as psS,
            tc.tile_pool(name="psC", bufs=2, space="PSUM") as psC,
            tc.tile_pool(name="psRB", bufs=2, space="PSUM") as psRB,
        ):
            for b in range(b_local):
                for hd in range(NH):
                    po = (hd % 2) * DH
                    sl = hd // 2
                    ssl = bass.ds(b * s, s)
                    qh = qT[po : po + DH, sl, ssl]
                    ets = []
                    for tci in range(SC):
                        ps = psS.tile([128, 512], F32, tag="psS")
                        nc.tensor.matmul(
                            ps,
                            lhsT=kT[po : po + DH, sl, bass.ds(b * s + tci * 128, 128)],
                            rhs=qh,
                            start=True,
                            stop=True,
                        )
                        et = expp.tile([128, 512], BF16, tag="exp")
                        nc.scalar.activation(
                            out=et, in_=ps, func=AF.Exp, scale=1.0 / 8.0
                        )
                        ets.append(et)
                    pc = psC.tile([VW, 512], F32, tag="psC")
                    for tci in range(SC):
                        mt = (b * s) // 128 + tci
                        nc.tensor.matmul(
                            pc,
                            lhsT=v_aug[:, mt, hd * VW : (hd + 1) * VW],
                            rhs=ets[tci],
                            start=(tci == 0),
                            stop=(tci == SC - 1),
                        )
                    # denominator lives on psum partition DH: reciprocal it
                    # straight out of PSUM into a bf16 row (keeps the K=1
                    # broadcast matmul at 1 cycle/row; f32 matmuls run at 4)
                    rrb = rrp.tile([DH + 1, 512], BF16, tag="rrb")
                    with nc.allow_low_precision(reason="softmax denom in bf16"):
                        nc.vector.reciprocal(rrb[DH : DH + 1, :], pc[DH : DH + 1, :])
                    # shift the reciprocal row to partition 0 (DMA), then
                    # broadcast across head partitions with a K=1 PE matmul
                    nc.gpsimd.dma_start(out=rrb[0:1, :], in_=rrb[DH : DH + 1, :])
                    ps_rb = psRB.tile([DH, 512], F32, tag="psRB")
                    nc.tensor.matmul(
                        ps_rb,
                        lhsT=ones_brow[:, 0:DH],
                        rhs=rrb[0:1, :],
                        start=True,
                        stop=True,
                    )
                    rb = rbp.tile([DH, 512], F32, tag="rb")
                    nc.vector.tensor_copy(out=rb, in_=ps_rb)
                    if po == 0:
                        nc.vector.tensor_tensor(
                            ctx[0:DH, sl, ssl], pc[0:DH, :], rb, op=ALU.mult
                        )
                    else:
                        tmp = rbp.tile([DH, 512], BF16, tag="ctmp")
                        nc.vector.tensor_tensor(tmp, pc[0:DH, :], rb, op=ALU.mult)
                        nc.gpsimd.dma_start(out=ctx[po : po + DH, sl, ssl], in_=tmp)

        # ---- phase C: output proj + residual + LN1 -----------------------
        x1 = big.tile([128, KH, T], F32, tag="bigbuf")
        with (
            tc.tile_pool(name="psO", bufs=3, space="PSUM") as psO,
            tc.tile_pool(name="psL1", bufs=3, space="PSUM") as psL1,
            tc.tile_pool(name="psB1", bufs=2, space="PSUM") as psB1,
        ):
            for m in range(KH):
                msl = bass.ts(m, 128)
                for n in range(NS):
                    tsl = bass.ts(n, 512)
                    ps = psO.tile([128, 512], F32, tag="psO")
                    for k in range(KH):
                        nc.tensor.matmul(
                            ps,
                            lhsT=wo_s[k][:, msl],
                            rhs=ctx[:, k, tsl],
                            start=(k == 0),
                            stop=(k == KH - 1),
                        )
                    nc.scalar.activation(
                        out=x1[:, m, tsl],
                        in_=ps,
                        func=AF.Identity,
                        bias=co_t[:, m : m + 1],
                    )
                    nc.vector.tensor_tensor(
                        x1[:, m, tsl], x1[:, m, tsl], h[:, m, tsl], op=ALU.add
                    )
            attn_bf = actb.tile([128, KH, T], BF16, tag="actb")
            layernorm(x1, cg1_t, cb1_t, attn_bf, psL1, psB1)
        attn = x1  # f32 LN1 output (post gain/bias)

        # ---- phase D: FFN (fused over I-chunks) + residual + LN2 ---------
        x2 = big.tile([128, KH, T], F32, tag="bigbuf")
        with (
            tc.tile_pool(name="psF", bufs=6, space="PSUM") as psF,
            tc.tile_pool(name="psG", bufs=2, space="PSUM") as psG,
        ):
            for n in range(NS):
                tsl = bass.ts(n, 512)
                outps = [psF.tile([128, 512], F32, tag="psF", name=f"outps{n}_{m}") for m in range(KH)]
                IG = 3  # i-chunks per weight-block DMA
                pend = None  # (gl, w2_blk, ii, i) awaiting its FFN2 pass
                for ig in range(KI // IG):
                    wi_blk = wib.tile(
                        [128, KH, IG * 128], BF16, tag="wib", name=f"wib{n}_{ig}"
                    )
                    nc.sync.dma_start(
                        out=wi_blk[:],
                        in_=wi[l, :, :, bass.ts(ig, IG * 128)].rearrange(
                            "k p w -> p k w"
                        ),
                    )
                    w2_blk = w2p.tile(
                        [128, IG, H], BF16, tag="w2", name=f"w2b{n}_{ig}"
                    )
                    nc.sync.dma_start(
                        out=w2_blk[:],
                        in_=wo2[l, ig * IG : (ig + 1) * IG].rearrange(
                            "i p h -> p i h"
                        ),
                    )
                    for ii in range(IG):
                        i = ig * IG + ii
                        psg = psG.tile([128, 512], F32, tag="psG")
                        for k in range(KH):
                            nc.tensor.matmul(
                                psg,
                                lhsT=wi_blk[:, k, bass.ts(ii, 128)],
                                rhs=attn_bf[:, k, tsl],
                                start=(k == 0),
                                stop=(k == KH - 1),
                            )
                        gl = gel.tile([128, 512], BF16, tag="gel")
                        nc.scalar.activation(
                            out=gl, in_=psg, func=AF.Gelu, bias=ci_t[:, i : i + 1]
                        )
                        if pend is not None:
                            pgl, pw2, pii, pi = pend
                            for m in range(KH):
                                nc.tensor.matmul(
                                    outps[m],
                                    lhsT=pw2[:, pii, bass.ts(m, 128)],
                                    rhs=pgl,
                                    start=(pi == 0),
                                    stop=False,
                                )
                        pend = (gl, w2_blk, ii, i)
                pgl, pw2, pii, pi = pend
                for m in range(KH):
                    nc.tensor.matmul(
                        outps[m],
                        lhsT=pw2[:, pii, bass.ts(m, 128)],
                        rhs=pgl,
                        start=False,
                        stop=True,
                    )
                for m in range(KH):
                    nc.scalar.activation(
                        out=x2[:, m, tsl],
                        in_=outps[m],
                        func=AF.Identity,
                        bias=co2_t[:, m : m + 1],
                    )
                    nc.vector.tensor_tensor(
                        x2[:, m, tsl], x2[:, m, tsl], attn[:, m, tsl], op=ALU.add
                    )
        out_bf = actb.tile([128, KH, T], BF16, tag="actb")
        with (
            tc.tile_pool(name="psL2", bufs=4, space="PSUM") as psL2,
            tc.tile_pool(name="psB2", bufs=2, space="PSUM") as psB2,
        ):
            layernorm(x2, cg2_t, cb2_t, out_bf, psL2, psB2, last=(l == n_layers - 1))
        h = x2
        h_bf = out_bf

    nc.sync.dma_start(out=outT, in_=h[:])
    ctx_stack.close()


# ---------------------------------------------------------------------------
# host side
# ---------------------------------------------------------------------------

BF = ml_dtypes.bfloat16
N_CORES = 8
B, S = 16, 512
B_LOCAL = B // N_CORES
N_LAYERS = 6


def _slabs(w, kdim):  # [L, in, out] -> [L, kdim, 128, out] bf16
    return np.ascontiguousarray(
        np.asarray(w, np.float32).reshape(N_LAYERS, kdim, 128, -1)
    ).astype(BF)


def _cols(b, kdim):  # [L, kdim*128] -> [L, 128, kdim] f32
    return np.ascontiguousarray(
        np.asarray(b, np.float32).reshape(N_LAYERS, kdim, 128).transpose(0, 2, 1)
    )


def _prep_shared(inputs):
    packed = np.concatenate(
        [
            _cols(inputs["bq"], KH),
            _cols(inputs["bk"], KH),
            _cols(inputs["bo"], KH),
            _cols(inputs["g1"], KH),
            _cols(inputs["b1"], KH),
            _cols(inputs["bo2"], KH),
            _cols(inputs["g2"], KH),
            _cols(inputs["b2"], KH),
            _cols(inputs["bi"], KI),
        ],
        axis=2,
    )
    return {
        "wq": _slabs(inputs["Wq"], KH),
        "wk": _slabs(inputs["Wk"], KH),
        "wv": _slabs(inputs["Wv"], KH),
        "wo": _slabs(inputs["Wo"], KH),
        "wi": _slabs(inputs["Wi"], KH),
        "wo2": _slabs(inputs["Wo2"], KI),
        "cols": np.ascontiguousarray(packed),
        "bvrow": np.ascontiguousarray(
            np.asarray(inputs["bv"], np.float32).reshape(N_LAYERS, 1, H)
        ),
    }


def _prep_hT(h_shard):
    b, s, _ = h_shard.shape
    T = b * s
    return np.ascontiguousarray(
        np.asarray(h_shard, np.float32)
        .reshape(T, H)
        .T.reshape(KH, 128, T)
        .transpose(1, 0, 2)
    )


def _unprep_outT(outT, b, s):
    T = b * s
    return np.ascontiguousarray(
        np.asarray(outT).transpose(1, 0, 2).reshape(H, T).T.reshape(b, s, H)
    )


_NC_CACHE = {}


def _get_nc():
    key = (N_LAYERS, B_LOCAL, S)
    if key not in _NC_CACHE:
        _NC_CACHE[key] = build_nc(N_LAYERS, B_LOCAL, S, num_devices=N_CORES)
    return _NC_CACHE[key]


def make_in_maps(**inputs):
    shared = _prep_shared(inputs)
    h = np.asarray(inputs["hidden_states"], np.float32)
    in_maps = []
    for c in range(N_CORES):
        m = dict(shared)
        m["hT"] = _prep_hT(h[c * B_LOCAL : (c + 1) * B_LOCAL])
        in_maps.append(m)
    return in_maps


def gather_output(results):
    return np.concatenate(
        [_unprep_outT(results[c]["outT"], B_LOCAL, S) for c in range(N_CORES)],
        axis=0,
    )


def kernel(**inputs):
    from concourse.bass_utils import run_bass_kernel_spmd

    nc = _get_nc()
    in_maps = make_in_maps(**inputs)
    res = run_bass_kernel_spmd(nc, in_maps, list(range(N_CORES)))
    return gather_output(res.results)



# revision 5
# speedup vs baseline: 3.7212x; 3.7212x over previous
"""Trainium2 Bass kernel for a 6-layer BERT encoder (nn_BertEncoder).

Strategy: data-parallel over batch across 8 NeuronCores (16 batches -> 2 per
core), no collectives.  Inside each core the encoder runs with feature-major
("transposed") activations so every projection uses the weight as the PE
stationary operand with zero transposes; attention computes transposed scores
so the softmax denominator falls out of an appended ones-column of V; layer
norm uses ones-vector matmuls for the partition reductions.  Matmuls run in
bf16 (f32 accumulation), layernorm/residual paths stay f32.
"""

import sys

sys.path.insert(0, "/opt/trn_rl_repo")

from contextlib import ExitStack

import numpy as np
import ml_dtypes

import concourse.bass as bass
import concourse.mybir as mybir
import concourse.tile as tile
from concourse.vector_clock import ScopedClock, VectorClock

F32 = mybir.dt.float32
F32R = mybir.dt.float32r
BF16 = mybir.dt.bfloat16
AF = mybir.ActivationFunctionType
ALU = mybir.AluOpType

H = 768
I = 3072
NH = 12
DH = 64
KH = H // 128  # 6 feature slabs
KI = I // 128  # 24 intermediate chunks
EPS = 1e-12


class SplitDrainTileContext(tile.TileContext):
    """TileContext whose kernel-tail drain splits its semaphore waits across
    several SP nops -- this walrus build rejects >1 sync wait on a TPB_CTRL
    (Drain/NoOp) instruction."""

    def _drain_and_barrier(self, tick_clock, wait_clock):
        gc = tick_clock.global_clock
        nprocs = len(gc)
        procs = [p for p in range(nprocs) if gc[p] > 0]
        covered = [0] * nprocs
        for p in procs:
            req = [0] * nprocs
            req[p] = gc[p]
            nop_inst = self.nc.sync.nop(nofuse=True)
            wait_clock.add_sem_waits(
                nop_inst.ins,
                ScopedClock({None: VectorClock(req)}),
                ScopedClock({None: VectorClock(list(covered))}),
            )
            covered[p] = gc[p]
        drain_inst = self.nc.sync.drain()
        wait_clock.add_sem_waits(
            drain_inst.ins,
            ScopedClock({None: gc.copy()}),
            ScopedClock({None: VectorClock(list(covered))}),
        )
        self.nc.all_engine_barrier()
        assert self.sems is not None
        popped = self.nc._tile_sem_poison_stack.pop()
        assert popped is self._sem_poison
        self.nc.clear_and_free_semaphores(list(self.sems.allocated().values()))
        self.nc.all_engine_barrier()




def split_multi_waits(nc):
    """Walrus in this container accepts at most ONE sync wait per
    instruction.  Split every instruction carrying N>1 waits into N-1
    same-engine NOPs (each holding one wait) followed by the original
    instruction with the final wait."""
    f = nc.m.functions[0]
    n_split = 0
    for bb in f.blocks:
        insts = list(bb.instructions)
        out = []
        changed = False
        for inst in insts:
            si = inst.sync_info
            if si is not None and len(si.on_wait) > 1:
                waits = list(si.on_wait)
                for j, w in enumerate(waits[:-1]):
                    nop = mybir.InstNoOp(
                        name=f"{inst.name}_sw{j}",
                        engine=inst.engine,
                        sync_info=mybir.SyncInfo(on_wait=[w], on_update=[]),
                        bass_nofuse=True,
                    )
                    out.append(nop)
                inst.sync_info = mybir.SyncInfo(
                    on_wait=[waits[-1]], on_update=list(si.on_update)
                )
                n_split += 1
                changed = True
            out.append(inst)
        if changed:
            bb.instructions = out
    # verify the mutation took effect (blocks could have been copies)
    for bb in nc.m.functions[0].blocks:
        for inst in bb.instructions:
            si = inst.sync_info
            assert si is None or len(si.on_wait) <= 1, (
                f"multi-wait survived on {inst.name}"
            )
    return n_split


def build_nc(n_layers: int, b_local: int, s: int, num_devices: int = 8, split_waits: bool = True, repeat: int = 1):
    """Build the per-core Bass module.  Per-core DRAM parameters:

      hT    [128, KH, T] f32      transposed hidden states
      wq/wk/wv/wo [L, KH, 128, H] bf16   weight k-slabs (in-major)
      wi    [L, KH, 128, I] bf16
      wo2   [L, KI, 128, H] bf16
      cq/ck/co/cg1/cb1/co2/cg2/cb2 [L, 128, KH] f32  per-partition bias cols
      ci    [L, 128, KI] f32
      bvrow [L, 1, H] f32
      outT  [128, KH, T] f32
    """
    T = b_local * s
    NT = T // 128  # token chunks
    NS = T // 512  # 512-wide token column blocks
    nc = bass.Bass(
        "TRN2", target_bir_lowering=False, debug=False, num_devices=num_devices
    )

    hT = nc.dram_tensor("hT", [128, KH, T], F32, kind="ExternalInput").ap()
    L = n_layers
    wq = nc.dram_tensor("wq", [L, KH, 128, H], BF16, kind="ExternalInput").ap()
    wk = nc.dram_tensor("wk", [L, KH, 128, H], BF16, kind="ExternalInput").ap()
    wv = nc.dram_tensor("wv", [L, KH, 128, H], BF16, kind="ExternalInput").ap()
    wo = nc.dram_tensor("wo", [L, KH, 128, H], BF16, kind="ExternalInput").ap()
    wi = nc.dram_tensor("wi", [L, KH, 128, I], BF16, kind="ExternalInput").ap()
    wo2 = nc.dram_tensor("wo2", [L, KI, 128, H], BF16, kind="ExternalInput").ap()
    # all per-partition bias/gain columns packed: 8 groups of KH + KI for ci
    cols = nc.dram_tensor(
        "cols", [L, 128, 8 * KH + KI], F32, kind="ExternalInput"
    ).ap()
    bvrow = nc.dram_tensor("bvrow", [L, 1, H], F32, kind="ExternalInput").ap()
    outT = nc.dram_tensor("outT", [128, KH, T], F32, kind="ExternalOutput").ap()

    with SplitDrainTileContext(nc) as tc:
        for _ in range(repeat):
            build_body(
                tc,
                n_layers,
                b_local,
                s,
                hT,
                (wq, wk, wv, wo, wi, wo2),
                (cols, bvrow),
                outT,
            )
    if split_waits:
        split_multi_waits(nc)
    return nc


_PHASE_MARKS = []  # (phase_label, first_inst_id) — analysis only


def _mark(nc, label):
    _PHASE_MARKS.append((label, nc.next_id()))


def build_body(tc, n_layers, b_local, s, hT, weights, biases, outT):
    nc = tc.nc
    T = b_local * s
    NT = T // 128
    NS = T // 512
    SC = s // 128  # score row chunks per (batch, head) = 4
    wq, wk, wv, wo, wi, wo2 = weights
    cols, bvrow = biases

    VW = DH + 1  # v_aug per-head width (64 + ones column)

    # ---- persistent pools (kept open for the whole kernel) ----------------
    ctx_stack = ExitStack()
    ec = ctx_stack.enter_context
    big = ec(tc.tile_pool(name="big_f32", bufs=2))  # x1 / x2+out
    actb = ec(tc.tile_pool(name="act_bf16", bufs=2))  # h/ctx/xsq/attn
    qkp = ec(tc.tile_pool(name="qk", bufs=2))
    vap = ec(tc.tile_pool(name="vaug", bufs=1))
    expp = ec(tc.tile_pool(name="exp", bufs=4))
    rbp = ec(tc.tile_pool(name="rbcast", bufs=2))
    lnb = ec(tc.tile_pool(name="lnbcast", bufs=2))
    rows = ec(tc.tile_pool(name="rows", bufs=2))
    rrp = ec(tc.tile_pool(name="rrp", bufs=2))
    wpj = ec(tc.tile_pool(name="wproj", bufs=4))
    wib = ec(tc.tile_pool(name="wiblk", bufs=2))
    w2p = ec(tc.tile_pool(name="wo2", bufs=2))
    bcol = ec(tc.tile_pool(name="bcols", bufs=2))
    gel = ec(tc.tile_pool(name="gelu", bufs=3))
    ones_pool = ec(tc.tile_pool(name="ones", bufs=1))

    ones_f32 = ones_pool.tile([128, 1], F32)
    nc.vector.memset(ones_f32, 1.0)
    ones_bf16 = ones_pool.tile([128, 1], BF16)
    nc.vector.memset(ones_bf16, 1.0)
    eps_t = ones_pool.tile([128, 1], F32)
    nc.vector.memset(eps_t, EPS)
    ones_f32row = ones_pool.tile([1, 128], F32)
    nc.vector.memset(ones_f32row, 1.0)
    ones_brow = ones_pool.tile([1, 128], BF16)
    nc.vector.memset(ones_brow, 1.0)

    # ---- load initial hidden state ---------------------------------------
    h = big.tile([128, KH, T], F32, tag="bigbuf")
    nc.sync.dma_start(out=h[:], in_=hT)
    h_bf = actb.tile([128, KH, T], BF16, tag="actb")
    nc.scalar.activation(out=h_bf[:], in_=h[:], func=AF.Copy)

    def layernorm(x, g_col, b_col, out_bf, psum_ln, psum_bc, last=False):
        """LN over features (partitions across KH slabs) of x [128,KH,T] f32.
        Applies in place (x <- LN(x)*g+b); writes bf16 copy to out_bf unless
        last.  Stats: bf16 rounded copy + square, summed over partitions with
        ones-vector matmuls; all row math stays on partition 0."""
        xb = actb.tile([128, KH, T], BF16, tag="actb")
        nc.scalar.activation(out=xb[:], in_=x[:], func=AF.Copy)
        xsq = actb.tile([128, KH, T], BF16, tag="actb")
        nc.vector.tensor_tensor(xsq[:], xb[:], xb[:], op=ALU.mult)
        srow = rows.tile([1, 3 * T], F32, tag="srow", bufs=1)
        mean_row = srow[:, 0:T]
        msq_row = srow[:, T : 2 * T]
        var_row = srow[:, 2 * T : 3 * T]
        for n in range(NS):
            tsl = bass.ts(n, 512)
            ps_sum = psum_ln.tile([1, 512], F32, tag="lnp")
            for k in range(KH):
                nc.tensor.matmul(
                    ps_sum,
                    lhsT=ones_bf16,
                    rhs=xb[:, k, tsl],
                    start=(k == 0),
                    stop=(k == KH - 1),
                )
            ps_sq = psum_ln.tile([1, 512], F32, tag="lnp")
            for k in range(KH):
                nc.tensor.matmul(
                    ps_sq,
                    lhsT=ones_bf16,
                    rhs=xsq[:, k, tsl],
                    start=(k == 0),
                    stop=(k == KH - 1),
                )
            nc.scalar.activation(
                out=mean_row[:, tsl], in_=ps_sum, func=AF.Copy, scale=1.0 / H
            )
            nc.scalar.activation(
                out=msq_row[:, tsl], in_=ps_sq, func=AF.Copy, scale=1.0 / H
            )
        nc.vector.tensor_tensor(var_row, mean_row, mean_row, op=ALU.mult)
        nc.vector.tensor_tensor(var_row, msq_row, var_row, op=ALU.subtract)
        nc.scalar.activation(out=var_row, in_=var_row, func=AF.Sqrt, bias=eps_t[0:1, :])
        nc.vector.reciprocal(var_row, var_row)
        rstd_row = var_row
        mean_b = lnb.tile([128, T], F32, tag="lnb")
        rstd_b = lnb.tile([128, T], F32, tag="lnb")
        for n in range(NS):
            tsl = bass.ts(n, 512)
            for row, dst in ((mean_row, mean_b), (rstd_row, rstd_b)):
                ps_bc = psum_bc.tile([128, 512], F32, tag="lnbc")
                nc.tensor.matmul(
                    ps_bc, lhsT=ones_f32row, rhs=row[:, tsl], start=True, stop=True
                )
                nc.scalar.activation(out=dst[:, tsl], in_=ps_bc, func=AF.Copy)
        for k in range(KH):
            nc.vector.tensor_tensor(x[:, k, :], x[:, k, :], mean_b, op=ALU.subtract)
            nc.vector.tensor_tensor(x[:, k, :], x[:, k, :], rstd_b, op=ALU.mult)
            # bf16 copy (with per-partition gain/bias) on ACT, f32 on DVE
            if not last:
                nc.scalar.activation(
                    out=out_bf[:, k, :],
                    in_=x[:, k, :],
                    func=AF.Identity,
                    scale=g_col[:, k : k + 1],
                    bias=b_col[:, k : k + 1],
                )
            nc.vector.tensor_scalar(
                out=x[:, k, :],
                in0=x[:, k, :],
                scalar1=g_col[:, k : k + 1],
                scalar2=b_col[:, k : k + 1],
                op0=ALU.mult,
                op1=ALU.add,
            )

    for l in range(n_layers):
        _mark(nc, f"L{l}_const")
        # ---- per-layer constants (single DMA for all bias columns) -------
        cols_t = bcol.tile([128, 8 * KH + KI], F32, tag="cols")
        nc.sync.dma_start(out=cols_t[:], in_=cols[l])
        cq_t = cols_t[:, 0 * KH : 1 * KH]
        ck_t = cols_t[:, 1 * KH : 2 * KH]
        co_t = cols_t[:, 2 * KH : 3 * KH]
        cg1_t = cols_t[:, 3 * KH : 4 * KH]
        cb1_t = cols_t[:, 4 * KH : 5 * KH]
        co2_t = cols_t[:, 5 * KH : 6 * KH]
        cg2_t = cols_t[:, 6 * KH : 7 * KH]
        cb2_t = cols_t[:, 7 * KH : 8 * KH]
        ci_t = cols_t[:, 8 * KH : 8 * KH + KI]
        bvbuf = rows.tile([1, max(T, H)], F32, tag="bvrow", bufs=1)
        bv_r = bvbuf[0:1, 0:H]
        nc.sync.dma_start(out=bv_r, in_=bvrow[l])
        bv_b = bcol.tile([128, H], F32, tag="bvb", bufs=1)

        wq_t = wpj.tile([128, KH, H], BF16, tag="wpj", name=f"wq_{l}")
        nc.sync.dma_start(out=wq_t[:], in_=wq[l].rearrange("k p h -> p k h"))
        wk_t = wpj.tile([128, KH, H], BF16, tag="wpj", name=f"wk_{l}")
        nc.sync.dma_start(out=wk_t[:], in_=wk[l].rearrange("k p h -> p k h"))
        wv_t = wpj.tile([128, KH, H], BF16, tag="wpj", name=f"wv_{l}")
        nc.sync.dma_start(out=wv_t[:], in_=wv[l].rearrange("k p h -> p k h"))
        wo_t = wpj.tile([128, KH, H], BF16, tag="wpj", name=f"wo_{l}")
        nc.sync.dma_start(out=wo_t[:], in_=wo[l].rearrange("k p h -> p k h"))
        wq_s = [wq_t[:, k, :] for k in range(KH)]
        wk_s = [wk_t[:, k, :] for k in range(KH)]
        wv_s = [wv_t[:, k, :] for k in range(KH)]
        wo_s = [wo_t[:, k, :] for k in range(KH)]

        _mark(nc, f"L{l}_A_qkv")
        # ---- phase A: QKV projections ------------------------------------
        qT = qkp.tile([128, KH, T], BF16, tag="qk")
        kT = qkp.tile([128, KH, T], BF16, tag="qk")
        v_aug = vap.tile([128, NT, NH * VW], BF16, tag="vaug")
        with tc.tile_pool(name="psA", bufs=6, space="PSUM") as psA:
            for f0, fw in ((0, 512), (512, 256)):
                ps_bv = psA.tile([128, 512], F32, tag="psA")
                nc.tensor.matmul(
                    ps_bv[:, 0:fw],
                    lhsT=ones_f32row,
                    rhs=bv_r[:, f0 : f0 + fw],
                    start=True,
                    stop=True,
                )
                nc.vector.tensor_copy(out=bv_b[:, f0 : f0 + fw], in_=ps_bv[:, 0:fw])
            for m in range(KH):
                for n in range(NS):
                    tsl = bass.ts(n, 512)
                    msl = bass.ts(m, 128)
                    ps = psA.tile([128, 512], F32, tag="psA")
                    for k in range(KH):
                        nc.tensor.matmul(
                            ps,
                            lhsT=wq_s[k][:, msl],
                            rhs=h_bf[:, k, tsl],
                            start=(k == 0),
                            stop=(k == KH - 1),
                        )
                    nc.scalar.activation(
                        out=qT[:, m, tsl],
                        in_=ps,
                        func=AF.Identity,
                        bias=cq_t[:, m : m + 1],
                    )
                    ps2 = psA.tile([128, 512], F32, tag="psA")
                    for k in range(KH):
                        nc.tensor.matmul(
                            ps2,
                            lhsT=wk_s[k][:, msl],
                            rhs=h_bf[:, k, tsl],
                            start=(k == 0),
                            stop=(k == KH - 1),
                        )
                    nc.scalar.activation(
                        out=kT[:, m, tsl],
                        in_=ps2,
                        func=AF.Identity,
                        bias=ck_t[:, m : m + 1],
                    )
            # V in natural layout (tokens on partitions), heads interleaved
            # with a ones column:  v_aug[:, mt, h*VW : h*VW+64] = v tokens x dh
            for mt in range(NT):
                tsl = bass.ts(mt, 128)
                nc.vector.memset(
                    v_aug[:, mt, :].rearrange("p (h d) -> p h d", d=VW)[:, :, DH:VW],
                    1.0,
                )
                for half, (f0, fw) in enumerate([(0, 512), (512, 256)]):
                    ps = psA.tile([128, 512], F32, tag="psA")
                    for k in range(KH):
                        nc.tensor.matmul(
                            ps[:, 0:fw],
                            lhsT=h_bf[:, k, tsl],
                            rhs=wv_s[k][:, f0 : f0 + fw],
                            start=(k == 0),
                            stop=(k == KH - 1),
                        )
                    nh0 = f0 // DH
                    nh = fw // DH
                    out_ap = v_aug[:, mt, nh0 * VW : (nh0 + nh) * VW].rearrange(
                        "p (h d) -> p h d", d=VW
                    )[:, :, 0:DH]
                    nc.vector.tensor_tensor(
                        out_ap,
                        ps[:, 0:fw].rearrange("p (h d) -> p h d", d=DH),
                        bv_b[:, f0 : f0 + fw].rearrange("p (h d) -> p h d", d=DH),
                        op=ALU.add,
                    )

        _mark(nc, f"L{l}_B_attn")
        # ---- phase B: attention ------------------------------------------
        ctx = actb.tile([128, KH, T], BF16, tag="actb")
        with (
            tc.tile_pool(name="psS", bufs=4, space="PSUM") as psS,
            tc.tile_pool(name="psC", bufs=2, space="PSUM") as psC,
            tc.tile_pool(name="psRB", bufs=2, space="PSUM") as psRB,
        ):
            for b in range(b_local):
                for hd in range(NH):
                    po = (hd % 2) * DH
                    sl = hd // 2
                    ssl = bass.ds(b * s, s)
                    qh = qT[po : po + DH, sl, ssl]
                    ets = []
                    for tci in range(SC):
                        ps = psS.tile([128, 512], F32, tag="psS")
                        nc.tensor.matmul(
                            ps,
                            lhsT=kT[po : po + DH, sl, bass.ds(b * s + tci * 128, 128)],
                            rhs=qh,
                            start=True,
                            stop=True,
                        )
                        et = expp.tile([128, 512], BF16, tag="exp")
                        nc.scalar.activation(
                            out=et, in_=ps, func=AF.Exp, scale=1.0 / 8.0
                        )
                        ets.append(et)
                    pc = psC.tile([VW, 512], F32, tag="psC")
                    for tci in range(SC):
                        mt = (b * s) // 128 + tci
                        nc.tensor.matmul(
                            pc,
                            lhsT=v_aug[:, mt, hd * VW : (hd + 1) * VW],
                            rhs=ets[tci],
                            start=(tci == 0),
                            stop=(tci == SC - 1),
                        )
                    # denominator lives on psum partition DH: reciprocal it
                    # straight out of PSUM into a bf16 row (keeps the K=1
                    # broadcast matmul at 1 cycle/row; f32 matmuls run at 4)
                    rrb = rrp.tile([DH + 1, 512], BF16, tag="rrb")
                    with nc.allow_low_precision(reason="softmax denom in bf16"):
                        nc.vector.reciprocal(rrb[DH : DH + 1, :], pc[DH : DH + 1, :])
                    # shift the reciprocal row to partition 0 (DMA), then
                    # broadcast across head partitions with a K=1 PE matmul
                    nc.gpsimd.dma_start(out=rrb[0:1, :], in_=rrb[DH : DH + 1, :])
                    ps_rb = psRB.tile([DH, 512], F32, tag="psRB")
                    nc.tensor.matmul(
                        ps_rb,
                        lhsT=ones_brow[:, 0:DH],
                        rhs=rrb[0:1, :],
                        start=True,
                        stop=True,
                    )
                    rb = rbp.tile([DH, 512], F32, tag="rb")
                    nc.vector.tensor_copy(out=rb, in_=ps_rb)
                    if po == 0:
                        nc.vector.tensor_tensor(
                            ctx[0:DH, sl, ssl], pc[0:DH, :], rb, op=ALU.mult
                        )
                    else:
                        tmp = rbp.tile([DH, 512], BF16, tag="ctmp")
                        nc.vector.tensor_tensor(tmp, pc[0:DH, :], rb, op=ALU.mult)
                        nc.gpsimd.dma_start(out=ctx[po : po + DH, sl, ssl], in_=tmp)

        _mark(nc, f"L{l}_C_oproj")
        # ---- phase C: output proj + residual + LN1 -----------------------
        x1 = big.tile([128, KH, T], F32, tag="bigbuf")
        with (
            tc.tile_pool(name="psO", bufs=3, space="PSUM") as psO,
            tc.tile_pool(name="psL1", bufs=3, space="PSUM") as psL1,
            tc.tile_pool(name="psB1", bufs=2, space="PSUM") as psB1,
        ):
            for m in range(KH):
                msl = bass.ts(m, 128)
                for n in range(NS):
                    tsl = bass.ts(n, 512)
                    ps = psO.tile([128, 512], F32, tag="psO")
                    for k in range(KH):
                        nc.tensor.matmul(
                            ps,
                            lhsT=wo_s[k][:, msl],
                            rhs=ctx[:, k, tsl],
                            start=(k == 0),
                            stop=(k == KH - 1),
                        )
                    nc.scalar.activation(
                        out=x1[:, m, tsl],
                        in_=ps,
                        func=AF.Identity,
                        bias=co_t[:, m : m + 1],
                    )
                    nc.vector.tensor_tensor(
                        x1[:, m, tsl], x1[:, m, tsl], h[:, m, tsl], op=ALU.add
                    )
            attn_bf = actb.tile([128, KH, T], BF16, tag="actb")
            layernorm(x1, cg1_t, cb1_t, attn_bf, psL1, psB1)
        attn = x1  # f32 LN1 output (post gain/bias)

        _mark(nc, f"L{l}_D_ffn")
        # ---- phase D: FFN (fused over I-chunks) + residual + LN2 ---------
        x2 = big.tile([128, KH, T], F32, tag="bigbuf")
        with (
            tc.tile_pool(name="psF", bufs=6, space="PSUM") as psF,
            tc.tile_pool(name="psG", bufs=2, space="PSUM") as psG,
        ):
            for n in range(NS):
                tsl = bass.ts(n, 512)
                outps = [psF.tile([128, 512], F32, tag="psF", name=f"outps{n}_{m}") for m in range(KH)]
                IG = 3  # i-chunks per weight-block DMA
                pend = None  # (gl, w2_blk, ii, i) awaiting its FFN2 pass
                for ig in range(KI // IG):
                    wi_blk = wib.tile(
                        [128, KH, IG * 128], BF16, tag="wib", name=f"wib{n}_{ig}"
                    )
                    nc.sync.dma_start(
                        out=wi_blk[:],
                        in_=wi[l, :, :, bass.ts(ig, IG * 128)].rearrange(
                            "k p w -> p k w"
                        ),
                    )
                    w2_blk = w2p.tile(
                        [128, IG, H], BF16, tag="w2", name=f"w2b{n}_{ig}"
                    )
                    nc.sync.dma_start(
                        out=w2_blk[:],
                        in_=wo2[l, ig * IG : (ig + 1) * IG].rearrange(
                            "i p h -> p i h"
                        ),
                    )
                    for ii in range(IG):
                        i = ig * IG + ii
                        psg = psG.tile([128, 512], F32, tag="psG")
                        for k in range(KH):
                            nc.tensor.matmul(
                                psg,
                                lhsT=wi_blk[:, k, bass.ts(ii, 128)],
                                rhs=attn_bf[:, k, tsl],
                                start=(k == 0),
                                stop=(k == KH - 1),
                            )
                        gl = gel.tile([128, 512], BF16, tag="gel")
                        nc.scalar.activation(
                            out=gl, in_=psg, func=AF.Gelu, bias=ci_t[:, i : i + 1]
                        )
                        if pend is not None:
                            pgl, pw2, pii, pi = pend
                            for m in range(KH):
                                nc.tensor.matmul(
                                    outps[m],
                                    lhsT=pw2[:, pii, bass.ts(m, 128)],
                                    rhs=pgl,
                                    start=(pi == 0),
                                    stop=False,
                                )
                        pend = (gl, w2_blk, ii, i)
                pgl, pw2, pii, pi = pend
                for m in range(KH):
                    nc.tensor.matmul(
                        outps[m],
                        lhsT=pw2[:, pii, bass.ts(m, 128)],
                        rhs=pgl,
                        start=False,
                        stop=True,
                    )
                for m in range(KH):
                    nc.scalar.activation(
                        out=x2[:, m, tsl],
                        in_=outps[m],
                        func=AF.Identity,
                        bias=co2_t[:, m : m + 1],
                    )
                    nc.vector.tensor_tensor(
                        x2[:, m, tsl], x2[:, m, tsl], attn[:, m, tsl], op=ALU.add
                    )
        _mark(nc, f"L{l}_E_ln2")
        out_bf = actb.tile([128, KH, T], BF16, tag="actb")
        with (
            tc.tile_pool(name="psL2", bufs=4, space="PSUM") as psL2,
            tc.tile_pool(name="psB2", bufs=2, space="PSUM") as psB2,
        ):
            layernorm(x2, cg2_t, cb2_t, out_bf, psL2, psB2, last=(l == n_layers - 1))
        h = x2
        h_bf = out_bf

    nc.sync.dma_start(out=outT, in_=h[:])
    ctx_stack.close()


# ---------------------------------------------------------------------------
# host side
# ---------------------------------------------------------------------------

BF = ml_dtypes.bfloat16
N_CORES = 8
B, S = 16, 512
B_LOCAL = B // N_CORES
N_LAYERS = 6


def _slabs(w, kdim):  # [L, in, out] -> [L, kdim, 128, out] bf16
    return np.ascontiguousarray(
        np.asarray(w, np.float32).reshape(N_LAYERS, kdim, 128, -1)
    ).astype(BF)


def _cols(b, kdim):  # [L, kdim*128] -> [L, 128, kdim] f32
    return np.ascontiguousarray(
        np.asarray(b, np.float32).reshape(N_LAYERS, kdim, 128).transpose(0, 2, 1)
    )


def _prep_shared(inputs):
    packed = np.concatenate(
        [
            _cols(inputs["bq"], KH),
            _cols(inputs["bk"], KH),
            _cols(inputs["bo"], KH),
            _cols(inputs["g1"], KH),
            _cols(inputs["b1"], KH),
            _cols(inputs["bo2"], KH),
            _cols(inputs["g2"], KH),
            _cols(inputs["b2"], KH),
            _cols(inputs["bi"], KI),
        ],
        axis=2,
    )
    return {
        "wq": _slabs(inputs["Wq"], KH),
        "wk": _slabs(inputs["Wk"], KH),
        "wv": _slabs(inputs["Wv"], KH),
        "wo": _slabs(inputs["Wo"], KH),
        "wi": _slabs(inputs["Wi"], KH),
        "wo2": _slabs(inputs["Wo2"], KI),
        "cols": np.ascontiguousarray(packed),
        "bvrow": np.ascontiguousarray(
            np.asarray(inputs["bv"], np.float32).reshape(N_LAYERS, 1, H)
        ),
    }


def _prep_hT(h_shard):
    b, s, _ = h_shard.shape
    T = b * s
    return np.ascontiguousarray(
        np.asarray(h_shard, np.float32)
        .reshape(T, H)
        .T.reshape(KH, 128, T)
        .transpose(1, 0, 2)
    )


def _unprep_outT(outT, b, s):
    T = b * s
    return np.ascontiguousarray(
        np.asarray(outT).transpose(1, 0, 2).reshape(H, T).T.reshape(b, s, H)
    )


_NC_CACHE = {}


def _get_nc():
    key = (N_LAYERS, B_LOCAL, S)
    if key not in _NC_CACHE:
        _NC_CACHE[key] = build_nc(N_LAYERS, B_LOCAL, S, num_devices=N_CORES)
    return _NC_CACHE[key]


def make_in_maps(**inputs):
    shared = _prep_shared(inputs)
    h = np.asarray(inputs["hidden_states"], np.float32)
    in_maps = []
    for c in range(N_CORES):
        m = dict(shared)
        m["hT"] = _prep_hT(h[c * B_LOCAL : (c + 1) * B_LOCAL])
        in_maps.append(m)
    return in_maps


def gather_output(results):
    return np.concatenate(
        [_unprep_outT(results[c]["outT"], B_LOCAL, S) for c in range(N_CORES)],
        axis=0,
    )


def kernel(**inputs):
    from concourse.bass_utils import run_bass_kernel_spmd

    nc = _get_nc()
    in_maps = make_in_maps(**inputs)
    res = run_bass_kernel_spmd(nc, in_maps, list(range(N_CORES)))
    return gather_output(res.results)



# revision 15
# speedup vs baseline: 12.7582x; 3.4285x over previous
"""Trainium2 Bass kernel for a 6-layer BERT encoder (nn_BertEncoder).

Strategy: data-parallel over batch across 8 NeuronCores (16 batches -> 2 per
core), no collectives.  Weights are embedded in the NEFF as Const tensors
(inline_tensor) so each call only ships the hidden states.  Inside each core
the encoder runs with feature-major ("transposed") activations so every
projection uses the weight as the PE stationary operand with zero transposes.

Attention: transposed scores per head; per head-pair the even head's softmax
denominator comes free from a ones-column appended to V, the odd head's from
M=1 ones matmuls into partition 96 of the even context bank.  tile_position
keyed matmuls broadcast the reciprocal rows straight into the right partition
halves, so the whole softmax tail is DMA-free and partition-aligned.

LayerNorm: per 512-token block, pipelined with the producing matmul phase;
the per-token scale/shift rows are broadcast to [128,512] PSUM tiles via
K=1 matmuls (a = rstd, b = -mean*rstd) and applied with two tensor-tensor
ops per slab, gain/bias folded into a per-partition tensor-scalar.
"""

import sys

sys.path.insert(0, "/opt/trn_rl_repo")

from contextlib import ExitStack

import numpy as np
import ml_dtypes

import concourse.bass as bass
import concourse.mybir as mybir
import concourse.tile as tile
from concourse.vector_clock import ScopedClock, VectorClock

F32 = mybir.dt.float32
BF16 = mybir.dt.bfloat16
AF = mybir.ActivationFunctionType
ALU = mybir.AluOpType

H = 768
I = 3072
NH = 12
DH = 64
KH = H // 128  # 6 feature slabs
KI = I // 128  # 24 intermediate chunks
NP = NH // 2  # 6 head pairs
PW = 2 * DH + 1  # per-pair v_aug width (even v + ones col + odd v)
EPS = 1e-12


class SplitDrainTileContext(tile.TileContext):
    """TileContext whose kernel-tail drain splits its semaphore waits across
    several SP nops -- this walrus build rejects >1 sync wait on a TPB_CTRL
    (Drain/NoOp) instruction."""

    def _drain_and_barrier(self, tick_clock, wait_clock):
        gc = tick_clock.global_clock
        nprocs = len(gc)
        procs = [p for p in range(nprocs) if gc[p] > 0]
        covered = [0] * nprocs
        for p in procs:
            req = [0] * nprocs
            req[p] = gc[p]
            nop_inst = self.nc.sync.nop(nofuse=True)
            wait_clock.add_sem_waits(
                nop_inst.ins,
                ScopedClock({None: VectorClock(req)}),
                ScopedClock({None: VectorClock(list(covered))}),
            )
            covered[p] = gc[p]
        drain_inst = self.nc.sync.drain()
        wait_clock.add_sem_waits(
            drain_inst.ins,
            ScopedClock({None: gc.copy()}),
            ScopedClock({None: VectorClock(list(covered))}),
        )
        self.nc.all_engine_barrier()
        assert self.sems is not None
        popped = self.nc._tile_sem_poison_stack.pop()
        assert popped is self._sem_poison
        self.nc.clear_and_free_semaphores(list(self.sems.allocated().values()))
        self.nc.all_engine_barrier()


def split_multi_waits(nc):
    """Walrus in this container accepts at most ONE sync wait per
    instruction.  Split every instruction carrying N>1 waits into N-1
    same-engine NOPs (each holding one wait) followed by the original
    instruction with the final wait."""
    f = nc.m.functions[0]
    n_split = 0
    for bb in f.blocks:
        insts = list(bb.instructions)
        out = []
        changed = False
        for inst in insts:
            si = inst.sync_info
            if si is not None and len(si.on_wait) > 1:
                waits = list(si.on_wait)
                for j, w in enumerate(waits[:-1]):
                    nop = mybir.InstNoOp(
                        name=f"{inst.name}_sw{j}",
                        engine=inst.engine,
                        sync_info=mybir.SyncInfo(on_wait=[w], on_update=[]),
                        bass_nofuse=True,
                    )
                    out.append(nop)
                inst.sync_info = mybir.SyncInfo(
                    on_wait=[waits[-1]], on_update=list(si.on_update)
                )
                n_split += 1
                changed = True
            out.append(inst)
        if changed:
            bb.instructions = out
    for bb in nc.m.functions[0].blocks:
        for inst in bb.instructions:
            si = inst.sync_info
            assert si is None or len(si.on_wait) <= 1, (
                f"multi-wait survived on {inst.name}"
            )
    return n_split


_PHASE_MARKS = []  # (phase_label, first_inst_id) — analysis only


def _mark(nc, label):
    _PHASE_MARKS.append((label, nc.next_id()))


def build_nc(
    n_layers: int,
    b_local: int,
    s: int,
    weights: dict,
    num_devices: int = 8,
    split_waits: bool = True,
    repeat: int = 1,
):
    """Build the per-core Bass module.  `weights` holds the prepped host
    arrays (see _prep_shared); they are embedded in the NEFF as Const
    tensors.  Only hT (transposed hidden states) is a per-call input."""
    T = b_local * s
    nc = bass.Bass(
        "TRN2", target_bir_lowering=False, debug=False, num_devices=num_devices
    )

    hT = nc.dram_tensor("hT", [128, KH, T], F32, kind="ExternalInput").ap()
    w = {k: nc.inline_tensor(v, name=k).ap() for k, v in weights.items()}
    outT = nc.dram_tensor("outT", [128, KH, T], F32, kind="ExternalOutput").ap()

    with SplitDrainTileContext(nc) as tc:
        carry = []
        for r in range(repeat):
            carry = build_body(
                tc, n_layers, b_local, s, hT, w, outT, carry, last_pass=(r == repeat - 1)
            )
    if split_waits:
        split_multi_waits(nc)
    return nc


def build_body(tc, n_layers, b_local, s, hT, w, outT, carry_pools=(), last_pass=True):
    nc = tc.nc
    T = b_local * s
    NT = T // 128
    NS = T // 512  # = b_local (one 512-token block per batch)
    SC = s // 128  # key chunks per batch = 4
    wq, wk, wv, wo, wi, wo2 = w["wq"], w["wk"], w["wv"], w["wo"], w["wi"], w["wo2"]
    cols, bvrow = w["cols"], w["bvrow"]

    ctx_stack = ExitStack()
    ec = ctx_stack.enter_context
    big = ec(tc.tile_pool(name="big_f32", bufs=2))  # h / x1 / x2 (f32)
    actb = ec(tc.tile_pool(name="act_bf16", bufs=3))  # h_bf/ctx/attn_bf
    qkp = ec(tc.tile_pool(name="qk", bufs=2))
    vap = ec(tc.tile_pool(name="vaug", bufs=1))
    expp = ec(tc.tile_pool(name="exp", bufs=10))
    rbp = ec(tc.tile_pool(name="rbcast", bufs=2))
    lnw = ec(tc.tile_pool(name="lnwork", bufs=1))
    rows = ec(tc.tile_pool(name="rows", bufs=2))
    rrp = ec(tc.tile_pool(name="rrp", bufs=2))
    wpj = ec(tc.tile_pool(name="wproj", bufs=3))
    wib = ec(tc.tile_pool(name="wiblk", bufs=2))
    w2p = ec(tc.tile_pool(name="wo2", bufs=2))
    bcol = ec(tc.tile_pool(name="bcols", bufs=2))
    gel = ec(tc.tile_pool(name="gelu", bufs=4))
    ones_pool = ec(tc.tile_pool(name="ones", bufs=1))

    ones_bf16 = ones_pool.tile([128, 1], BF16)
    nc.vector.memset(ones_bf16, 1.0)
    eps_t = ones_pool.tile([128, 1], F32)
    nc.vector.memset(eps_t, EPS)
    ones_f32row = ones_pool.tile([1, 128], F32)
    nc.vector.memset(ones_f32row, 1.0)
    neg_f32row = ones_pool.tile([1, 128], F32)
    nc.vector.memset(neg_f32row, -1.0)
    # bf16 ones block: rows 64 and 96 feed the K=1 reciprocal broadcasts
    ones_blk = ones_pool.tile([128, DH], BF16)
    nc.vector.memset(ones_blk, 1.0)

    # ---- initial hidden state -------------------------------------------
    h = big.tile([128, KH, T], F32, tag="bigbuf")
    nc.sync.dma_start(out=h[:], in_=hT)
    h_bf = actb.tile([128, KH, T], BF16, tag="actb")
    nc.scalar.activation(out=h_bf[:], in_=h[:], func=AF.Copy)

    def ln_block(x, n, g_col, b_col, out_bf, psrows, psbc, last=False):
        """LayerNorm of token block n (512 cols) of x [128,KH,T] f32 in
        place; bf16 copy (with gain/bias) to out_bf unless last."""
        tsl = bass.ts(n, 512)
        xb = lnw.tile([128, KH, 512], BF16, tag="lnxb")
        nc.scalar.activation(out=xb[:], in_=x[:, :, tsl], func=AF.Copy)
        xsq = lnw.tile([128, KH, 512], BF16, tag="lnxsq")
        nc.vector.tensor_tensor(xsq[:], xb[:], xb[:], op=ALU.mult)
        ps_sum = psrows.tile([1, 512], F32, tag="lnrow")
        for k in range(KH):
            nc.tensor.matmul(
                ps_sum, lhsT=ones_bf16, rhs=xb[:, k, :],
                start=(k == 0), stop=(k == KH - 1),
            )
        ps_sq = psrows.tile([1, 512], F32, tag="lnrow")
        for k in range(KH):
            nc.tensor.matmul(
                ps_sq, lhsT=ones_bf16, rhs=xsq[:, k, :],
                start=(k == 0), stop=(k == KH - 1),
            )
        srow = rows.tile([1, 3 * 512], F32, tag="srow", bufs=1)
        mean_r = srow[:, 0:512]
        msq_r = srow[:, 512:1024]
        var_r = srow[:, 1024:1536]
        mr_r = mean_r  # mean*rstd overwrites the mean slot in place
        nc.scalar.activation(out=mean_r, in_=ps_sum, func=AF.Copy, scale=1.0 / H)
        nc.scalar.activation(out=msq_r, in_=ps_sq, func=AF.Copy, scale=1.0 / H)
        nc.vector.tensor_tensor(var_r, mean_r, mean_r, op=ALU.mult)
        nc.vector.tensor_tensor(var_r, msq_r, var_r, op=ALU.subtract)
        # rstd = 1/sqrt(var + eps); reuse msq slot for rstd
        rstd_r = msq_r
        nc.scalar.activation(out=var_r, in_=var_r, func=AF.Sqrt, bias=eps_t[0:1, :])
        nc.vector.reciprocal(rstd_r, var_r)
        nc.vector.tensor_tensor(mr_r, mean_r, rstd_r, op=ALU.mult)
        # broadcast a = rstd, b = -mean*rstd to [128,512] PSUM tiles
        ps_a = psbc.tile([128, 512], F32, tag="lnbc")
        nc.tensor.matmul(ps_a, lhsT=ones_f32row, rhs=rstd_r, start=True, stop=True)
        ps_b = psbc.tile([128, 512], F32, tag="lnbc")
        nc.tensor.matmul(ps_b, lhsT=neg_f32row, rhs=mr_r, start=True, stop=True)
        for k in range(KH):
            xs = x[:, k, tsl]
            nc.vector.tensor_tensor(xs, xs, ps_a, op=ALU.mult)
            nc.vector.tensor_tensor(xs, xs, ps_b, op=ALU.add)
            if not last:
                nc.scalar.activation(
                    out=out_bf[:, k, tsl],
                    in_=xs,
                    func=AF.Identity,
                    scale=g_col[:, k : k + 1],
                    bias=b_col[:, k : k + 1],
                )
            nc.vector.tensor_scalar(
                out=xs,
                in0=xs,
                scalar1=g_col[:, k : k + 1],
                scalar2=b_col[:, k : k + 1],
                op0=ALU.mult,
                op1=ALU.add,
            )

    pend_ln_pools = list(carry_pools)  # prev LN2 pools, closed after next QKV

    for l in range(n_layers):
        _mark(nc, f"L{l}_const")
        cols_t = bcol.tile([128, 8 * KH + KI], F32, tag="cols")
        nc.sync.dma_start(out=cols_t[:], in_=cols[l])
        cq_t = cols_t[:, 0 * KH : 1 * KH]
        ck_t = cols_t[:, 1 * KH : 2 * KH]
        co_t = cols_t[:, 2 * KH : 3 * KH]
        cg1_t = cols_t[:, 3 * KH : 4 * KH]
        cb1_t = cols_t[:, 4 * KH : 5 * KH]
        co2_t = cols_t[:, 5 * KH : 6 * KH]
        cg2_t = cols_t[:, 6 * KH : 7 * KH]
        cb2_t = cols_t[:, 7 * KH : 8 * KH]
        ci_t = cols_t[:, 8 * KH : 8 * KH + KI]
        bvbuf = rows.tile([1, H], F32, tag="bvrow", bufs=1)
        bv_r = bvbuf[0:1, 0:H]
        nc.sync.dma_start(out=bv_r, in_=bvrow[l])
        bv_b = bcol.tile([128, H], F32, tag="bvb", bufs=1)

        wq_t = wpj.tile([128, KH, H], BF16, tag="wpj", name=f"wq_{l}")
        nc.sync.dma_start(out=wq_t[:], in_=wq[l].rearrange("k p h -> p k h"))
        wk_t = wpj.tile([128, KH, H], BF16, tag="wpj", name=f"wk_{l}")
        nc.sync.dma_start(out=wk_t[:], in_=wk[l].rearrange("k p h -> p k h"))
        wv_t = wpj.tile([128, KH, H], BF16, tag="wpj", name=f"wv_{l}")
        nc.sync.dma_start(out=wv_t[:], in_=wv[l].rearrange("k p h -> p k h"))
        wo_t = wpj.tile([128, KH, H], BF16, tag="wpj", name=f"wo_{l}")
        nc.sync.dma_start(out=wo_t[:], in_=wo[l].rearrange("k p h -> p k h"))
        wq_s = [wq_t[:, k, :] for k in range(KH)]
        wk_s = [wk_t[:, k, :] for k in range(KH)]
        wv_s = [wv_t[:, k, :] for k in range(KH)]
        wo_s = [wo_t[:, k, :] for k in range(KH)]

        # ---- phase A: QKV projections -----------------------------------
        _mark(nc, f"L{l}_A_qkv")
        qT = qkp.tile([128, KH, T], BF16, tag="qk")
        kT = qkp.tile([128, KH, T], BF16, tag="qk")
        v_aug = vap.tile([128, NT, NP * PW], BF16, tag="vaug")
        psA_cm = tc.tile_pool(name=f"psA_{l}", bufs=4, space="PSUM")
        psA = psA_cm.__enter__()
        for f0, fw in ((0, 512), (512, 256)):
            ps_bv = psA.tile([128, 512], F32, tag="psA")
            nc.tensor.matmul(
                ps_bv[:, 0:fw],
                lhsT=ones_f32row,
                rhs=bv_r[:, f0 : f0 + fw],
                start=True,
                stop=True,
            )
            nc.vector.tensor_copy(out=bv_b[:, f0 : f0 + fw], in_=ps_bv[:, 0:fw])
        for n in range(NS):
            tsl = bass.ts(n, 512)
            for m in range(KH):
                msl = bass.ts(m, 128)
                ps = psA.tile([128, 512], F32, tag="psA")
                for k in range(KH):
                    nc.tensor.matmul(
                        ps,
                        lhsT=wq_s[k][:, msl],
                        rhs=h_bf[:, k, tsl],
                        start=(k == 0),
                        stop=(k == KH - 1),
                    )
                nc.scalar.activation(
                    out=qT[:, m, tsl], in_=ps, func=AF.Identity,
                    bias=cq_t[:, m : m + 1],
                )
                ps2 = psA.tile([128, 512], F32, tag="psA")
                for k in range(KH):
                    nc.tensor.matmul(
                        ps2,
                        lhsT=wk_s[k][:, msl],
                        rhs=h_bf[:, k, tsl],
                        start=(k == 0),
                        stop=(k == KH - 1),
                    )
                nc.scalar.activation(
                    out=kT[:, m, tsl], in_=ps2, func=AF.Identity,
                    bias=ck_t[:, m : m + 1],
                )
        # V in natural layout (tokens on partitions); per pair:
        # [even v (64) | ones col | odd v (64)]
        for mt in range(NT):
            tsl = bass.ts(mt, 128)
            vv = v_aug[:, mt, :].rearrange("p (pr w) -> p pr w", w=PW)
            nc.vector.memset(vv[:, :, DH : DH + 1], 1.0)
            for f0, fw in ((0, 512), (512, 256)):
                npr = fw // 128  # pairs in this feature range
                pr0 = f0 // 128
                ps = psA.tile([128, 512], F32, tag="psA")
                for k in range(KH):
                    nc.tensor.matmul(
                        ps[:, 0:fw],
                        lhsT=h_bf[:, k, tsl],
                        rhs=wv_s[k][:, f0 : f0 + fw],
                        start=(k == 0),
                        stop=(k == KH - 1),
                    )
                psv = ps[:, 0:fw].rearrange("p (pr two d) -> p pr two d", two=2, d=DH)
                bvv = bv_b[:, f0 : f0 + fw].rearrange(
                    "p (pr two d) -> p pr two d", two=2, d=DH
                )
                dst = v_aug[:, mt, pr0 * PW : (pr0 + npr) * PW].rearrange(
                    "p (pr w) -> p pr w", w=PW
                )
                nc.vector.tensor_tensor(
                    dst[:, :, 0:DH], psv[:, :, 0, :], bvv[:, :, 0, :], op=ALU.add
                )
                nc.vector.tensor_tensor(
                    dst[:, :, DH + 1 : PW], psv[:, :, 1, :], bvv[:, :, 1, :],
                    op=ALU.add,
                )
        psA_cm.__exit__(None, None, None)
        # close previous layer's LN2 psum pools (LIFO: they were entered
        # before psA) — QKV above overlaps the tail of the previous LN2
        for cm in reversed(pend_ln_pools):
            cm.__exit__(None, None, None)
        pend_ln_pools = []

        # ---- phase B: attention -----------------------------------------
        _mark(nc, f"L{l}_B_attn")
        ctx = actb.tile([128, KH, T], BF16, tag="actb")
        psO_cm = tc.tile_pool(name=f"psO_{l}", bufs=2, space="PSUM")
        psO = psO_cm.__enter__()
        psS_cm = tc.tile_pool(name=f"psS_{l}", bufs=3, space="PSUM")
        psS = psS_cm.__enter__()
        psC_cm = tc.tile_pool(name=f"psC_{l}", bufs=3, space="PSUM")
        psC = psC_cm.__enter__()
        for b in range(b_local):
            ssl = bass.ds(b * s, s)
            for pr in range(NP):
                sl = pr
                qh_e = qT[0:DH, sl, ssl]
                qh_o = qT[DH:128, sl, ssl]
                ets = {0: [], 1: []}  # parity -> [et [128,512] bf16]
                for par, qh in ((0, qh_e), (1, qh_o)):
                    kv = kT[par * DH : par * DH + DH, sl, :]
                    for tci in range(SC):
                        ps = psS.tile([128, 512], F32, tag="psS")
                        nc.tensor.matmul(
                            ps,
                            lhsT=kv[:, bass.ds(b * s + tci * 128, 128)],
                            rhs=qh,
                            start=True,
                            stop=True,
                        )
                        et = expp.tile([128, 512], BF16, tag="exp")
                        nc.scalar.activation(
                            out=et, in_=ps, func=AF.Exp, scale=1.0 / 8.0
                        )
                        ets[par].append(et)
                p0 = pr * PW
                pc_e = psC.tile([128, 512], F32, tag="psC")
                for tci in range(SC):
                    mt = b * SC + tci
                    nc.tensor.matmul(
                        pc_e[0 : DH + 1],
                        lhsT=v_aug[:, mt, p0 : p0 + DH + 1],
                        rhs=ets[0][tci],
                        start=(tci == 0),
                        stop=(tci == SC - 1),
                    )
                pc_o = psC.tile([128, 512], F32, tag="psC")
                for tci in range(SC):
                    mt = b * SC + tci
                    nc.tensor.matmul(
                        pc_o[DH:128],
                        lhsT=v_aug[:, mt, p0 + DH + 1 : p0 + PW],
                        rhs=ets[1][tci],
                        start=(tci == 0),
                        stop=(tci == SC - 1),
                    )
                # odd-head denominator into partition 96 of the even bank
                for tci in range(SC):
                    nc.tensor.matmul(
                        pc_e[96:97],
                        lhsT=ones_bf16,
                        rhs=ets[1][tci],
                        start=(tci == 0),
                        stop=(tci == SC - 1),
                        tile_position=(0, 96),
                        skip_group_check=True,
                    )
                # reciprocal rows (partitions 64 and 96), bf16
                rrb = rrp.tile([128, 512], BF16, tag="rrb")
                with nc.allow_low_precision(reason="softmax denom in bf16"):
                    nc.vector.reciprocal(rrb[DH : DH + 1, :], pc_e[DH : DH + 1, :])
                    nc.vector.reciprocal(rrb[96:97, :], pc_e[96:97, :])
                # broadcast 1/denom to the head's partition half
                pcb = psC.tile([128, 512], F32, tag="psC")
                nc.tensor.matmul(
                    pcb[0:DH],
                    lhsT=ones_blk[DH : DH + 1, :],
                    rhs=rrb[DH : DH + 1, :],
                    start=True,
                    stop=True,
                )
                nc.tensor.matmul(
                    pcb[DH:128],
                    lhsT=ones_blk[96:97, :],
                    rhs=rrb[96:97, :],
                    start=True,
                    stop=True,
                    tile_position=(96, DH),
                    skip_group_check=True,
                )
                rb = rbp.tile([128, 512], F32, tag="rb")
                nc.vector.tensor_copy(out=rb[0:DH], in_=pcb[0:DH])
                nc.vector.tensor_copy(out=rb[DH:128], in_=pcb[DH:128])
                nc.vector.tensor_tensor(
                    ctx[0:DH, sl, ssl], pc_e[0:DH], rb[0:DH], op=ALU.mult
                )
                nc.vector.tensor_tensor(
                    ctx[DH:128, sl, ssl], pc_o[DH:128], rb[DH:128], op=ALU.mult
                )
        psC_cm.__exit__(None, None, None)
        psS_cm.__exit__(None, None, None)
        # psO stays open: oproj(0) overlaps attn(1); exits after LN1

        # ---- phase C: output proj + residual + LN1 (per block) ----------
        _mark(nc, f"L{l}_C_oproj")
        x1 = big.tile([128, KH, T], F32, tag="bigbuf")
        attn_bf = actb.tile([128, KH, T], BF16, tag="actb")
        psL1r_cm = tc.tile_pool(name=f"psL1r_{l}", bufs=2, space="PSUM")
        psL1r = psL1r_cm.__enter__()
        psL1b_cm = tc.tile_pool(name=f"psL1b_{l}", bufs=2, space="PSUM")
        psL1b = psL1b_cm.__enter__()
        for n in range(NS):
            tsl = bass.ts(n, 512)
            for m in range(KH):
                msl = bass.ts(m, 128)
                ps = psO.tile([128, 512], F32, tag="psO")
                for k in range(KH):
                    nc.tensor.matmul(
                        ps,
                        lhsT=wo_s[k][:, msl],
                        rhs=ctx[:, k, tsl],
                        start=(k == 0),
                        stop=(k == KH - 1),
                    )
                nc.scalar.activation(
                    out=x1[:, m, tsl], in_=ps, func=AF.Identity,
                    bias=co_t[:, m : m + 1],
                )
                nc.vector.tensor_tensor(
                    x1[:, m, tsl], x1[:, m, tsl], h[:, m, tsl], op=ALU.add
                )
            ln_block(x1, n, cg1_t, cb1_t, attn_bf, psL1r, psL1b)
        psL1b_cm.__exit__(None, None, None)
        psL1r_cm.__exit__(None, None, None)
        psO_cm.__exit__(None, None, None)
        psG_cm = tc.tile_pool(name=f"psG_{l}", bufs=2, space="PSUM")
        psG = psG_cm.__enter__()

        # ---- phase D: FFN (fused over I-chunks) + residual --------------
        _mark(nc, f"L{l}_D_ffn")
        x2 = big.tile([128, KH, T], F32, tag="bigbuf")
        psF_cm = tc.tile_pool(name=f"psF_{l}", bufs=6, space="PSUM")
        psF = psF_cm.__enter__()
        for n in range(NS):
            tsl = bass.ts(n, 512)
            outps = [
                psF.tile([128, 512], F32, tag="psF", name=f"outps{n}_{m}")
                for m in range(KH)
            ]
            IG = 2  # i-chunks per weight-block DMA
            pend = None
            for ig in range(KI // IG):
                wi_blk = wib.tile(
                    [128, KH, IG * 128], BF16, tag="wib", name=f"wib{n}_{ig}"
                )
                nc.sync.dma_start(
                    out=wi_blk[:],
                    in_=wi[l, :, :, bass.ts(ig, IG * 128)].rearrange(
                        "k p w -> p k w"
                    ),
                )
                w2_blk = w2p.tile([128, IG, H], BF16, tag="w2", name=f"w2b{n}_{ig}")
                nc.sync.dma_start(
                    out=w2_blk[:],
                    in_=wo2[l, ig * IG : (ig + 1) * IG].rearrange("i p h -> p i h"),
                )
                for ii in range(IG):
                    i = ig * IG + ii
                    psg = psG.tile([128, 512], F32, tag="psG")
                    for k in range(KH):
                        nc.tensor.matmul(
                            psg,
                            lhsT=wi_blk[:, k, bass.ts(ii, 128)],
                            rhs=attn_bf[:, k, tsl],
                            start=(k == 0),
                            stop=(k == KH - 1),
                        )
                    gl = gel.tile([128, 512], BF16, tag="gel")
                    nc.scalar.activation(
                        out=gl, in_=psg, func=AF.Gelu, bias=ci_t[:, i : i + 1]
                    )
                    if pend is not None:
                        pgl, pw2, pii, pi = pend
                        for m in range(KH):
                            nc.tensor.matmul(
                                outps[m],
                                lhsT=pw2[:, pii, bass.ts(m, 128)],
                                rhs=pgl,
                                start=(pi == 0),
                                stop=False,
                            )
                    pend = (gl, w2_blk, ii, i)
            pgl, pw2, pii, pi = pend
            for m in range(KH):
                nc.tensor.matmul(
                    outps[m],
                    lhsT=pw2[:, pii, bass.ts(m, 128)],
                    rhs=pgl,
                    start=False,
                    stop=True,
                )
            for m in range(KH):
                nc.scalar.activation(
                    out=x2[:, m, tsl], in_=outps[m], func=AF.Identity,
                    bias=co2_t[:, m : m + 1],
                )
                nc.vector.tensor_tensor(
                    x2[:, m, tsl], x2[:, m, tsl], x1[:, m, tsl], op=ALU.add
                )

        # ---- phase E: LN2 (per block, overlaps next layer's QKV) --------
        _mark(nc, f"L{l}_E_ln2")
        last = l == n_layers - 1
        out_bf = None
        if not last:
            out_bf = actb.tile([128, KH, T], BF16, tag="actb")
        psF_cm.__exit__(None, None, None)
        psG_cm.__exit__(None, None, None)
        psL2r_cm = tc.tile_pool(name=f"psL2r_{l}", bufs=2, space="PSUM")
        psL2r = psL2r_cm.__enter__()
        psL2b_cm = tc.tile_pool(name=f"psL2b_{l}", bufs=2, space="PSUM")
        psL2b = psL2b_cm.__enter__()
        for n in range(NS):
            ln_block(x2, n, cg2_t, cb2_t, out_bf, psL2r, psL2b, last=last)
        pend_ln_pools = [psL2r_cm, psL2b_cm]
        h = x2
        h_bf = out_bf

    nc.sync.dma_start(out=outT, in_=h[:])
    if last_pass:
        for cm in reversed(pend_ln_pools):
            cm.__exit__(None, None, None)
        pend_ln_pools = []
    ctx_stack.close()
    return pend_ln_pools


# ---------------------------------------------------------------------------
# host side
# ---------------------------------------------------------------------------

BF = ml_dtypes.bfloat16
N_CORES = 8
B, S = 16, 512
B_LOCAL = B // N_CORES
N_LAYERS = 6


def _slabs(w, kdim):  # [L, in, out] -> [L, kdim, 128, out] bf16
    return np.ascontiguousarray(
        np.asarray(w, np.float32).reshape(N_LAYERS, kdim, 128, -1)
    ).astype(BF)


def _cols(b, kdim):  # [L, kdim*128] -> [L, 128, kdim] f32
    return np.ascontiguousarray(
        np.asarray(b, np.float32).reshape(N_LAYERS, kdim, 128).transpose(0, 2, 1)
    )


def _prep_shared(inputs):
    packed = np.concatenate(
        [
            _cols(inputs["bq"], KH),
            _cols(inputs["bk"], KH),
            _cols(inputs["bo"], KH),
            _cols(inputs["g1"], KH),
            _cols(inputs["b1"], KH),
            _cols(inputs["bo2"], KH),
            _cols(inputs["g2"], KH),
            _cols(inputs["b2"], KH),
            _cols(inputs["bi"], KI),
        ],
        axis=2,
    )
    return {
        "wq": _slabs(inputs["Wq"], KH),
        "wk": _slabs(inputs["Wk"], KH),
        "wv": _slabs(inputs["Wv"], KH),
        "wo": _slabs(inputs["Wo"], KH),
        "wi": _slabs(inputs["Wi"], KH),
        "wo2": _slabs(inputs["Wo2"], KI),
        "cols": np.ascontiguousarray(packed),
        "bvrow": np.ascontiguousarray(
            np.asarray(inputs["bv"], np.float32).reshape(N_LAYERS, 1, H)
        ),
    }


def _prep_hT(h_shard):
    b, s, _ = h_shard.shape
    T = b * s
    return np.ascontiguousarray(
        np.asarray(h_shard, np.float32)
        .reshape(T, H)
        .T.reshape(KH, 128, T)
        .transpose(1, 0, 2)
    )


def _unprep_outT(outT, b, s):
    T = b * s
    return np.ascontiguousarray(
        np.asarray(outT).transpose(1, 0, 2).reshape(H, T).T.reshape(b, s, H)
    )


_NC_CACHE = {}


def _weights_key(shared):
    h = 0
    for k in sorted(shared):
        a = shared[k]
        h ^= hash((k, a.shape, a.dtype.str, a.tobytes()[:256], a.tobytes()[-256:]))
    return h


def _get_nc(shared, repeat=1):
    key = (N_LAYERS, B_LOCAL, S, repeat, _weights_key(shared))
    if key not in _NC_CACHE:
        _NC_CACHE[key] = build_nc(
            N_LAYERS, B_LOCAL, S, shared, num_devices=N_CORES, repeat=repeat
        )
    return _NC_CACHE[key]


def make_in_maps(**inputs):
    h = np.asarray(inputs["hidden_states"], np.float32)
    return [
        {"hT": _prep_hT(h[c * B_LOCAL : (c + 1) * B_LOCAL])} for c in range(N_CORES)
    ]


def gather_output(results):
    return np.concatenate(
        [_unprep_outT(results[c]["outT"], B_LOCAL, S) for c in range(N_CORES)],
        axis=0,
    )


def kernel(**inputs):
    from concourse.bass_utils import run_bass_kernel_spmd

    shared = _prep_shared(inputs)
    nc = _get_nc(shared)
    in_maps = make_in_maps(**inputs)
    res = run_bass_kernel_spmd(nc, in_maps, list(range(N_CORES)))
    return gather_output(res.results)


# revision 16
# speedup vs baseline: 16.6208x; 1.3028x over previous
"""Trainium2 Bass kernel for a 6-layer BERT encoder (nn_BertEncoder).

Strategy: data-parallel over batch across 8 NeuronCores (16 batches -> 2 per
core), no collectives.  Weights are embedded in the NEFF as Const tensors
(inline_tensor) so each call only ships the hidden states.  Inside each core
the encoder runs with feature-major ("transposed") activations so every
projection uses the weight as the PE stationary operand with zero transposes.

Attention: transposed scores per head; per head-pair the even head's softmax
denominator comes free from a ones-column appended to V, the odd head's from
M=1 ones matmuls into partition 96 of the even context bank.  tile_position
keyed matmuls broadcast the reciprocal rows straight into the right partition
halves, so the whole softmax tail is DMA-free and partition-aligned.

LayerNorm: per 512-token block, pipelined with the producing matmul phase;
the per-token scale/shift rows are broadcast to [128,512] PSUM tiles via
K=1 matmuls (a = rstd, b = -mean*rstd) and applied with two tensor-tensor
ops per slab, gain/bias folded into a per-partition tensor-scalar.
"""

import sys

sys.path.insert(0, "/opt/trn_rl_repo")

from contextlib import ExitStack

import numpy as np
import ml_dtypes

import concourse.bass as bass
import concourse.mybir as mybir
import concourse.tile as tile
from concourse.vector_clock import ScopedClock, VectorClock

F32 = mybir.dt.float32
BF16 = mybir.dt.bfloat16
AF = mybir.ActivationFunctionType
ALU = mybir.AluOpType

H = 768
I = 3072
NH = 12
DH = 64
KH = H // 128  # 6 feature slabs
KI = I // 128  # 24 intermediate chunks
NP = NH // 2  # 6 head pairs
PW = 2 * DH + 1  # per-pair v_aug width (even v + ones col + odd v)
EPS = 1e-12


class SplitDrainTileContext(tile.TileContext):
    """TileContext whose kernel-tail drain splits its semaphore waits across
    several SP nops -- this walrus build rejects >1 sync wait on a TPB_CTRL
    (Drain/NoOp) instruction."""

    def _drain_and_barrier(self, tick_clock, wait_clock):
        gc = tick_clock.global_clock
        nprocs = len(gc)
        procs = [p for p in range(nprocs) if gc[p] > 0]
        covered = [0] * nprocs
        for p in procs:
            req = [0] * nprocs
            req[p] = gc[p]
            nop_inst = self.nc.sync.nop(nofuse=True)
            wait_clock.add_sem_waits(
                nop_inst.ins,
                ScopedClock({None: VectorClock(req)}),
                ScopedClock({None: VectorClock(list(covered))}),
            )
            covered[p] = gc[p]
        drain_inst = self.nc.sync.drain()
        wait_clock.add_sem_waits(
            drain_inst.ins,
            ScopedClock({None: gc.copy()}),
            ScopedClock({None: VectorClock(list(covered))}),
        )
        self.nc.all_engine_barrier()
        assert self.sems is not None
        popped = self.nc._tile_sem_poison_stack.pop()
        assert popped is self._sem_poison
        self.nc.clear_and_free_semaphores(list(self.sems.allocated().values()))
        self.nc.all_engine_barrier()


def split_multi_waits(nc):
    """Walrus in this container accepts at most ONE sync wait per
    instruction.  Split every instruction carrying N>1 waits into N-1
    same-engine NOPs (each holding one wait) followed by the original
    instruction with the final wait."""
    f = nc.m.functions[0]
    n_split = 0
    for bb in f.blocks:
        insts = list(bb.instructions)
        out = []
        changed = False
        for inst in insts:
            si = inst.sync_info
            if si is not None and len(si.on_wait) > 1:
                waits = list(si.on_wait)
                for j, w in enumerate(waits[:-1]):
                    nop = mybir.InstNoOp(
                        name=f"{inst.name}_sw{j}",
                        engine=inst.engine,
                        sync_info=mybir.SyncInfo(on_wait=[w], on_update=[]),
                        bass_nofuse=True,
                    )
                    out.append(nop)
                inst.sync_info = mybir.SyncInfo(
                    on_wait=[waits[-1]], on_update=list(si.on_update)
                )
                n_split += 1
                changed = True
            out.append(inst)
        if changed:
            bb.instructions = out
    for bb in nc.m.functions[0].blocks:
        for inst in bb.instructions:
            si = inst.sync_info
            assert si is None or len(si.on_wait) <= 1, (
                f"multi-wait survived on {inst.name}"
            )
    return n_split


_PHASE_MARKS = []  # (phase_label, first_inst_id) — analysis only


def _mark(nc, label):
    _PHASE_MARKS.append((label, nc.next_id()))


def build_nc(
    n_layers: int,
    b_local: int,
    s: int,
    weights: dict,
    num_devices: int = 8,
    split_waits: bool = True,
    repeat: int = 1,
):
    """Build the per-core Bass module.  `weights` holds the prepped host
    arrays (see _prep_shared); they are embedded in the NEFF as Const
    tensors.  Only hT (transposed hidden states) is a per-call input."""
    T = b_local * s
    nc = bass.Bass(
        "TRN2", target_bir_lowering=False, debug=False, num_devices=num_devices
    )

    hT = nc.dram_tensor("hT", [128, KH, T], F32, kind="ExternalInput").ap()
    w = {k: nc.inline_tensor(v, name=k).ap() for k, v in weights.items()}
    outT = nc.dram_tensor("outT", [128, KH, T], F32, kind="ExternalOutput").ap()

    with SplitDrainTileContext(nc) as tc:
        carry = []
        for r in range(repeat):
            carry = build_body(
                tc, n_layers, b_local, s, hT, w, outT, carry, last_pass=(r == repeat - 1)
            )
    if split_waits:
        split_multi_waits(nc)
    return nc


def build_body(tc, n_layers, b_local, s, hT, w, outT, carry_pools=(), last_pass=True):
    nc = tc.nc
    T = b_local * s
    NT = T // 128
    NS = T // 512  # = b_local (one 512-token block per batch)
    SC = s // 128  # key chunks per batch = 4
    wq, wk, wv, wo, wi, wo2 = w["wq"], w["wk"], w["wv"], w["wo"], w["wi"], w["wo2"]
    cols, bvrow = w["cols"], w["bvrow"]

    ctx_stack = ExitStack()
    ec = ctx_stack.enter_context
    big = ec(tc.tile_pool(name="big_f32", bufs=2))  # h / x1 / x2 (f32)
    actb = ec(tc.tile_pool(name="act_bf16", bufs=3))  # h_bf/ctx/attn_bf
    qkp = ec(tc.tile_pool(name="qk", bufs=2))
    vap = ec(tc.tile_pool(name="vaug", bufs=1))
    expp = ec(tc.tile_pool(name="exp", bufs=10))
    rbp = ec(tc.tile_pool(name="rbcast", bufs=2))
    lnw = ec(tc.tile_pool(name="lnwork", bufs=1))
    rows = ec(tc.tile_pool(name="rows", bufs=2))
    rrp = ec(tc.tile_pool(name="rrp", bufs=2))
    wpj = ec(tc.tile_pool(name="wproj", bufs=3))
    wib = ec(tc.tile_pool(name="wiblk", bufs=2))
    w2p = ec(tc.tile_pool(name="wo2", bufs=2))
    bcol = ec(tc.tile_pool(name="bcols", bufs=2))
    gel = ec(tc.tile_pool(name="gelu", bufs=4))
    ones_pool = ec(tc.tile_pool(name="ones", bufs=1))

    ones_bf16 = ones_pool.tile([128, 1], BF16)
    nc.vector.memset(ones_bf16, 1.0)
    eps_t = ones_pool.tile([128, 1], F32)
    nc.vector.memset(eps_t, EPS)
    ones_f32row = ones_pool.tile([1, 128], F32)
    nc.vector.memset(ones_f32row, 1.0)
    neg_f32row = ones_pool.tile([1, 128], F32)
    nc.vector.memset(neg_f32row, -1.0)
    # bf16 ones block: rows 64 and 96 feed the K=1 reciprocal broadcasts
    ones_blk = ones_pool.tile([128, DH], BF16)
    nc.vector.memset(ones_blk, 1.0)

    # ---- initial hidden state -------------------------------------------
    h = big.tile([128, KH, T], F32, tag="bigbuf")
    nc.sync.dma_start(out=h[:], in_=hT)
    h_bf = actb.tile([128, KH, T], BF16, tag="actb")
    nc.scalar.activation(out=h_bf[:], in_=h[:], func=AF.Copy)

    def ln_block(x, n, g_col, b_col, out_bf, psrows, psbc, last=False):
        """LayerNorm of token block n (512 cols) of x [128,KH,T] f32 in
        place; bf16 copy (with gain/bias) to out_bf unless last."""
        tsl = bass.ts(n, 512)
        xb = lnw.tile([128, KH, 512], BF16, tag="lnxb")
        nc.scalar.activation(out=xb[:], in_=x[:, :, tsl], func=AF.Copy)
        xsq = lnw.tile([128, KH, 512], BF16, tag="lnxsq")
        nc.vector.tensor_tensor(xsq[:], xb[:], xb[:], op=ALU.mult)
        ps_sum = psrows.tile([1, 512], F32, tag="lnrow")
        for k in range(KH):
            nc.tensor.matmul(
                ps_sum, lhsT=ones_bf16, rhs=xb[:, k, :],
                start=(k == 0), stop=(k == KH - 1),
            )
        ps_sq = psrows.tile([1, 512], F32, tag="lnrow")
        for k in range(KH):
            nc.tensor.matmul(
                ps_sq, lhsT=ones_bf16, rhs=xsq[:, k, :],
                start=(k == 0), stop=(k == KH - 1),
            )
        srow = rows.tile([1, 3 * 512], F32, tag="srow", bufs=1)
        mean_r = srow[:, 0:512]
        msq_r = srow[:, 512:1024]
        var_r = srow[:, 1024:1536]
        mr_r = mean_r  # mean*rstd overwrites the mean slot in place
        nc.scalar.activation(out=mean_r, in_=ps_sum, func=AF.Copy, scale=1.0 / H)
        nc.scalar.activation(out=msq_r, in_=ps_sq, func=AF.Copy, scale=1.0 / H)
        nc.vector.tensor_tensor(var_r, mean_r, mean_r, op=ALU.mult)
        nc.vector.tensor_tensor(var_r, msq_r, var_r, op=ALU.subtract)
        # rstd = exp(-0.5*ln(var+eps)): stays in the natural_log_exp ACT
        # table set (shared with attention's Exp) -- avoids the ~2.7us
        # sqrt table-set switch twice per layer
        rstd_r = msq_r
        nc.scalar.activation(out=var_r, in_=var_r, func=AF.Ln, bias=eps_t[0:1, :])
        nc.scalar.activation(out=rstd_r, in_=var_r, func=AF.Exp, scale=-0.5)
        nc.vector.tensor_tensor(mr_r, mean_r, rstd_r, op=ALU.mult)
        # broadcast a = rstd, b = -mean*rstd to [128,512] PSUM tiles
        ps_a = psbc.tile([128, 512], F32, tag="lnbc")
        nc.tensor.matmul(ps_a, lhsT=ones_f32row, rhs=rstd_r, start=True, stop=True)
        ps_b = psbc.tile([128, 512], F32, tag="lnbc")
        nc.tensor.matmul(ps_b, lhsT=neg_f32row, rhs=mr_r, start=True, stop=True)
        for k in range(KH):
            xs = x[:, k, tsl]
            nc.vector.tensor_tensor(xs, xs, ps_a, op=ALU.mult)
            nc.vector.tensor_tensor(xs, xs, ps_b, op=ALU.add)
            if not last:
                nc.scalar.activation(
                    out=out_bf[:, k, tsl],
                    in_=xs,
                    func=AF.Identity,
                    scale=g_col[:, k : k + 1],
                    bias=b_col[:, k : k + 1],
                )
            nc.vector.tensor_scalar(
                out=xs,
                in0=xs,
                scalar1=g_col[:, k : k + 1],
                scalar2=b_col[:, k : k + 1],
                op0=ALU.mult,
                op1=ALU.add,
            )

    pend_ln_pools = list(carry_pools)  # prev LN2 pools, closed after next QKV

    for l in range(n_layers):
        _mark(nc, f"L{l}_const")
        cols_t = bcol.tile([128, 8 * KH + KI], F32, tag="cols")
        nc.sync.dma_start(out=cols_t[:], in_=cols[l])
        cq_t = cols_t[:, 0 * KH : 1 * KH]
        ck_t = cols_t[:, 1 * KH : 2 * KH]
        co_t = cols_t[:, 2 * KH : 3 * KH]
        cg1_t = cols_t[:, 3 * KH : 4 * KH]
        cb1_t = cols_t[:, 4 * KH : 5 * KH]
        co2_t = cols_t[:, 5 * KH : 6 * KH]
        cg2_t = cols_t[:, 6 * KH : 7 * KH]
        cb2_t = cols_t[:, 7 * KH : 8 * KH]
        ci_t = cols_t[:, 8 * KH : 8 * KH + KI]
        bvbuf = rows.tile([1, H], F32, tag="bvrow", bufs=1)
        bv_r = bvbuf[0:1, 0:H]
        nc.sync.dma_start(out=bv_r, in_=bvrow[l])
        bv_b = bcol.tile([128, H], F32, tag="bvb", bufs=1)

        wq_t = wpj.tile([128, KH, H], BF16, tag="wpj", name=f"wq_{l}")
        nc.sync.dma_start(out=wq_t[:], in_=wq[l].rearrange("k p h -> p k h"))
        wk_t = wpj.tile([128, KH, H], BF16, tag="wpj", name=f"wk_{l}")
        nc.sync.dma_start(out=wk_t[:], in_=wk[l].rearrange("k p h -> p k h"))
        wv_t = wpj.tile([128, KH, H], BF16, tag="wpj", name=f"wv_{l}")
        nc.sync.dma_start(out=wv_t[:], in_=wv[l].rearrange("k p h -> p k h"))
        wo_t = wpj.tile([128, KH, H], BF16, tag="wpj", name=f"wo_{l}")
        nc.sync.dma_start(out=wo_t[:], in_=wo[l].rearrange("k p h -> p k h"))
        wq_s = [wq_t[:, k, :] for k in range(KH)]
        wk_s = [wk_t[:, k, :] for k in range(KH)]
        wv_s = [wv_t[:, k, :] for k in range(KH)]
        wo_s = [wo_t[:, k, :] for k in range(KH)]

        # ---- phase A: QKV projections -----------------------------------
        _mark(nc, f"L{l}_A_qkv")
        qT = qkp.tile([128, KH, T], BF16, tag="qk")
        kT = qkp.tile([128, KH, T], BF16, tag="qk")
        v_aug = vap.tile([128, NT, NP * PW], BF16, tag="vaug")
        psA_cm = tc.tile_pool(name=f"psA_{l}", bufs=4, space="PSUM")
        psA = psA_cm.__enter__()
        for f0, fw in ((0, 512), (512, 256)):
            ps_bv = psA.tile([128, 512], F32, tag="psA")
            nc.tensor.matmul(
                ps_bv[:, 0:fw],
                lhsT=ones_f32row,
                rhs=bv_r[:, f0 : f0 + fw],
                start=True,
                stop=True,
            )
            nc.vector.tensor_copy(out=bv_b[:, f0 : f0 + fw], in_=ps_bv[:, 0:fw])
        for n in range(NS):
            tsl = bass.ts(n, 512)
            for m in range(KH):
                msl = bass.ts(m, 128)
                ps = psA.tile([128, 512], F32, tag="psA")
                for k in range(KH):
                    nc.tensor.matmul(
                        ps,
                        lhsT=wq_s[k][:, msl],
                        rhs=h_bf[:, k, tsl],
                        start=(k == 0),
                        stop=(k == KH - 1),
                    )
                nc.scalar.activation(
                    out=qT[:, m, tsl], in_=ps, func=AF.Identity,
                    bias=cq_t[:, m : m + 1],
                )
                ps2 = psA.tile([128, 512], F32, tag="psA")
                for k in range(KH):
                    nc.tensor.matmul(
                        ps2,
                        lhsT=wk_s[k][:, msl],
                        rhs=h_bf[:, k, tsl],
                        start=(k == 0),
                        stop=(k == KH - 1),
                    )
                nc.scalar.activation(
                    out=kT[:, m, tsl], in_=ps2, func=AF.Identity,
                    bias=ck_t[:, m : m + 1],
                )
        # V in natural layout (tokens on partitions); per pair:
        # [even v (64) | ones col | odd v (64)]
        for mt in range(NT):
            tsl = bass.ts(mt, 128)
            vv = v_aug[:, mt, :].rearrange("p (pr w) -> p pr w", w=PW)
            nc.vector.memset(vv[:, :, DH : DH + 1], 1.0)
            for f0, fw in ((0, 512), (512, 256)):
                npr = fw // 128  # pairs in this feature range
                pr0 = f0 // 128
                ps = psA.tile([128, 512], F32, tag="psA")
                for k in range(KH):
                    nc.tensor.matmul(
                        ps[:, 0:fw],
                        lhsT=h_bf[:, k, tsl],
                        rhs=wv_s[k][:, f0 : f0 + fw],
                        start=(k == 0),
                        stop=(k == KH - 1),
                    )
                psv = ps[:, 0:fw].rearrange("p (pr two d) -> p pr two d", two=2, d=DH)
                bvv = bv_b[:, f0 : f0 + fw].rearrange(
                    "p (pr two d) -> p pr two d", two=2, d=DH
                )
                dst = v_aug[:, mt, pr0 * PW : (pr0 + npr) * PW].rearrange(
                    "p (pr w) -> p pr w", w=PW
                )
                nc.vector.tensor_tensor(
                    dst[:, :, 0:DH], psv[:, :, 0, :], bvv[:, :, 0, :], op=ALU.add
                )
                nc.vector.tensor_tensor(
                    dst[:, :, DH + 1 : PW], psv[:, :, 1, :], bvv[:, :, 1, :],
                    op=ALU.add,
                )
        psA_cm.__exit__(None, None, None)
        # close previous layer's LN2 psum pools (LIFO: they were entered
        # before psA) — QKV above overlaps the tail of the previous LN2
        for cm in reversed(pend_ln_pools):
            cm.__exit__(None, None, None)
        pend_ln_pools = []

        # ---- phase B: attention -----------------------------------------
        _mark(nc, f"L{l}_B_attn")
        ctx = actb.tile([128, KH, T], BF16, tag="actb")
        psO_cm = tc.tile_pool(name=f"psO_{l}", bufs=2, space="PSUM")
        psO = psO_cm.__enter__()
        psS_cm = tc.tile_pool(name=f"psS_{l}", bufs=3, space="PSUM")
        psS = psS_cm.__enter__()
        psC_cm = tc.tile_pool(name=f"psC_{l}", bufs=3, space="PSUM")
        psC = psC_cm.__enter__()
        for b in range(b_local):
            ssl = bass.ds(b * s, s)
            for pr in range(NP):
                sl = pr
                qh_e = qT[0:DH, sl, ssl]
                qh_o = qT[DH:128, sl, ssl]
                ets = {0: [], 1: []}  # parity -> [et [128,512] bf16]
                for par, qh in ((0, qh_e), (1, qh_o)):
                    kv = kT[par * DH : par * DH + DH, sl, :]
                    for tci in range(SC):
                        ps = psS.tile([128, 512], F32, tag="psS")
                        nc.tensor.matmul(
                            ps,
                            lhsT=kv[:, bass.ds(b * s + tci * 128, 128)],
                            rhs=qh,
                            start=True,
                            stop=True,
                        )
                        et = expp.tile([128, 512], BF16, tag="exp")
                        nc.scalar.activation(
                            out=et, in_=ps, func=AF.Exp, scale=1.0 / 8.0
                        )
                        ets[par].append(et)
                p0 = pr * PW
                pc_e = psC.tile([128, 512], F32, tag="psC")
                for tci in range(SC):
                    mt = b * SC + tci
                    nc.tensor.matmul(
                        pc_e[0 : DH + 1],
                        lhsT=v_aug[:, mt, p0 : p0 + DH + 1],
                        rhs=ets[0][tci],
                        start=(tci == 0),
                        stop=(tci == SC - 1),
                    )
                pc_o = psC.tile([128, 512], F32, tag="psC")
                for tci in range(SC):
                    mt = b * SC + tci
                    nc.tensor.matmul(
                        pc_o[DH:128],
                        lhsT=v_aug[:, mt, p0 + DH + 1 : p0 + PW],
                        rhs=ets[1][tci],
                        start=(tci == 0),
                        stop=(tci == SC - 1),
                    )
                # odd-head denominator into partition 96 of the even bank
                for tci in range(SC):
                    nc.tensor.matmul(
                        pc_e[96:97],
                        lhsT=ones_bf16,
                        rhs=ets[1][tci],
                        start=(tci == 0),
                        stop=(tci == SC - 1),
                        tile_position=(0, 96),
                        skip_group_check=True,
                    )
                # reciprocal rows (partitions 64 and 96), bf16
                rrb = rrp.tile([128, 512], BF16, tag="rrb")
                with nc.allow_low_precision(reason="softmax denom in bf16"):
                    nc.vector.reciprocal(rrb[DH : DH + 1, :], pc_e[DH : DH + 1, :])
                    nc.vector.reciprocal(rrb[96:97, :], pc_e[96:97, :])
                # broadcast 1/denom to the head's partition half
                pcb = psC.tile([128, 512], F32, tag="psC")
                nc.tensor.matmul(
                    pcb[0:DH],
                    lhsT=ones_blk[DH : DH + 1, :],
                    rhs=rrb[DH : DH + 1, :],
                    start=True,
                    stop=True,
                )
                nc.tensor.matmul(
                    pcb[DH:128],
                    lhsT=ones_blk[96:97, :],
                    rhs=rrb[96:97, :],
                    start=True,
                    stop=True,
                    tile_position=(96, DH),
                    skip_group_check=True,
                )
                rb = rbp.tile([128, 512], F32, tag="rb")
                nc.vector.tensor_copy(out=rb[0:DH], in_=pcb[0:DH])
                nc.vector.tensor_copy(out=rb[DH:128], in_=pcb[DH:128])
                nc.vector.tensor_tensor(
                    ctx[0:DH, sl, ssl], pc_e[0:DH], rb[0:DH], op=ALU.mult
                )
                nc.vector.tensor_tensor(
                    ctx[DH:128, sl, ssl], pc_o[DH:128], rb[DH:128], op=ALU.mult
                )
        psC_cm.__exit__(None, None, None)
        psS_cm.__exit__(None, None, None)
        # psO stays open: oproj(0) overlaps attn(1); exits after LN1

        # ---- phase C: output proj + residual + LN1 (per block) ----------
        _mark(nc, f"L{l}_C_oproj")
        x1 = big.tile([128, KH, T], F32, tag="bigbuf")
        attn_bf = actb.tile([128, KH, T], BF16, tag="actb")
        psL1r_cm = tc.tile_pool(name=f"psL1r_{l}", bufs=2, space="PSUM")
        psL1r = psL1r_cm.__enter__()
        psL1b_cm = tc.tile_pool(name=f"psL1b_{l}", bufs=2, space="PSUM")
        psL1b = psL1b_cm.__enter__()
        for n in range(NS):
            tsl = bass.ts(n, 512)
            for m in range(KH):
                msl = bass.ts(m, 128)
                ps = psO.tile([128, 512], F32, tag="psO")
                for k in range(KH):
                    nc.tensor.matmul(
                        ps,
                        lhsT=wo_s[k][:, msl],
                        rhs=ctx[:, k, tsl],
                        start=(k == 0),
                        stop=(k == KH - 1),
                    )
                nc.scalar.activation(
                    out=x1[:, m, tsl], in_=ps, func=AF.Identity,
                    bias=co_t[:, m : m + 1],
                )
                nc.vector.tensor_tensor(
                    x1[:, m, tsl], x1[:, m, tsl], h[:, m, tsl], op=ALU.add
                )
            ln_block(x1, n, cg1_t, cb1_t, attn_bf, psL1r, psL1b)
        psL1b_cm.__exit__(None, None, None)
        psL1r_cm.__exit__(None, None, None)
        psO_cm.__exit__(None, None, None)
        psG_cm = tc.tile_pool(name=f"psG_{l}", bufs=2, space="PSUM")
        psG = psG_cm.__enter__()

        # ---- phase D: FFN (fused over I-chunks) + residual --------------
        _mark(nc, f"L{l}_D_ffn")
        x2 = big.tile([128, KH, T], F32, tag="bigbuf")
        psF_cm = tc.tile_pool(name=f"psF_{l}", bufs=6, space="PSUM")
        psF = psF_cm.__enter__()
        for n in range(NS):
            tsl = bass.ts(n, 512)
            outps = [
                psF.tile([128, 512], F32, tag="psF", name=f"outps{n}_{m}")
                for m in range(KH)
            ]
            IG = 2  # i-chunks per weight-block DMA
            pend = None
            for ig in range(KI // IG):
                wi_blk = wib.tile(
                    [128, KH, IG * 128], BF16, tag="wib", name=f"wib{n}_{ig}"
                )
                nc.sync.dma_start(
                    out=wi_blk[:],
                    in_=wi[l, :, :, bass.ts(ig, IG * 128)].rearrange(
                        "k p w -> p k w"
                    ),
                )
                w2_blk = w2p.tile([128, IG, H], BF16, tag="w2", name=f"w2b{n}_{ig}")
                nc.sync.dma_start(
                    out=w2_blk[:],
                    in_=wo2[l, ig * IG : (ig + 1) * IG].rearrange("i p h -> p i h"),
                )
                for ii in range(IG):
                    i = ig * IG + ii
                    psg = psG.tile([128, 512], F32, tag="psG")
                    for k in range(KH):
                        nc.tensor.matmul(
                            psg,
                            lhsT=wi_blk[:, k, bass.ts(ii, 128)],
                            rhs=attn_bf[:, k, tsl],
                            start=(k == 0),
                            stop=(k == KH - 1),
                        )
                    gl = gel.tile([128, 512], BF16, tag="gel")
                    nc.scalar.activation(
                        out=gl, in_=psg, func=AF.Gelu, bias=ci_t[:, i : i + 1]
                    )
                    if pend is not None:
                        pgl, pw2, pii, pi = pend
                        for m in range(KH):
                            nc.tensor.matmul(
                                outps[m],
                                lhsT=pw2[:, pii, bass.ts(m, 128)],
                                rhs=pgl,
                                start=(pi == 0),
                                stop=False,
                            )
                    pend = (gl, w2_blk, ii, i)
            pgl, pw2, pii, pi = pend
            for m in range(KH):
                nc.tensor.matmul(
                    outps[m],
                    lhsT=pw2[:, pii, bass.ts(m, 128)],
                    rhs=pgl,
                    start=False,
                    stop=True,
                )
            for m in range(KH):
                nc.scalar.activation(
                    out=x2[:, m, tsl], in_=outps[m], func=AF.Identity,
                    bias=co2_t[:, m : m + 1],
                )
                nc.vector.tensor_tensor(
                    x2[:, m, tsl], x2[:, m, tsl], x1[:, m, tsl], op=ALU.add
                )

        # ---- phase E: LN2 (per block, overlaps next layer's QKV) --------
        _mark(nc, f"L{l}_E_ln2")
        last = l == n_layers - 1
        out_bf = None
        if not last:
            out_bf = actb.tile([128, KH, T], BF16, tag="actb")
        psF_cm.__exit__(None, None, None)
        psG_cm.__exit__(None, None, None)
        psL2r_cm = tc.tile_pool(name=f"psL2r_{l}", bufs=2, space="PSUM")
        psL2r = psL2r_cm.__enter__()
        psL2b_cm = tc.tile_pool(name=f"psL2b_{l}", bufs=2, space="PSUM")
        psL2b = psL2b_cm.__enter__()
        for n in range(NS):
            ln_block(x2, n, cg2_t, cb2_t, out_bf, psL2r, psL2b, last=last)
        pend_ln_pools = [psL2r_cm, psL2b_cm]
        h = x2
        h_bf = out_bf

    nc.sync.dma_start(out=outT, in_=h[:])
    if last_pass:
        for cm in reversed(pend_ln_pools):
            cm.__exit__(None, None, None)
        pend_ln_pools = []
    ctx_stack.close()
    return pend_ln_pools


# ---------------------------------------------------------------------------
# host side
# ---------------------------------------------------------------------------

BF = ml_dtypes.bfloat16
N_CORES = 8
B, S = 16, 512
B_LOCAL = B // N_CORES
N_LAYERS = 6


def _slabs(w, kdim):  # [L, in, out] -> [L, kdim, 128, out] bf16
    return np.ascontiguousarray(
        np.asarray(w, np.float32).reshape(N_LAYERS, kdim, 128, -1)
    ).astype(BF)


def _cols(b, kdim):  # [L, kdim*128] -> [L, 128, kdim] f32
    return np.ascontiguousarray(
        np.asarray(b, np.float32).reshape(N_LAYERS, kdim, 128).transpose(0, 2, 1)
    )


def _prep_shared(inputs):
    packed = np.concatenate(
        [
            _cols(inputs["bq"], KH),
            _cols(inputs["bk"], KH),
            _cols(inputs["bo"], KH),
            _cols(inputs["g1"], KH),
            _cols(inputs["b1"], KH),
            _cols(inputs["bo2"], KH),
            _cols(inputs["g2"], KH),
            _cols(inputs["b2"], KH),
            _cols(inputs["bi"], KI),
        ],
        axis=2,
    )
    return {
        "wq": _slabs(inputs["Wq"], KH),
        "wk": _slabs(inputs["Wk"], KH),
        "wv": _slabs(inputs["Wv"], KH),
        "wo": _slabs(inputs["Wo"], KH),
        "wi": _slabs(inputs["Wi"], KH),
        "wo2": _slabs(inputs["Wo2"], KI),
        "cols": np.ascontiguousarray(packed),
        "bvrow": np.ascontiguousarray(
            np.asarray(inputs["bv"], np.float32).reshape(N_LAYERS, 1, H)
        ),
    }


def _prep_hT(h_shard):
    b, s, _ = h_shard.shape
    T = b * s
    return np.ascontiguousarray(
        np.asarray(h_shard, np.float32)
        .reshape(T, H)
        .T.reshape(KH, 128, T)
        .transpose(1, 0, 2)
    )


def _unprep_outT(outT, b, s):
    T = b * s
    return np.ascontiguousarray(
        np.asarray(outT).transpose(1, 0, 2).reshape(H, T).T.reshape(b, s, H)
    )


_NC_CACHE = {}


def _weights_key(shared):
    h = 0
    for k in sorted(shared):
        a = shared[k]
        h ^= hash((k, a.shape, a.dtype.str, a.tobytes()[:256], a.tobytes()[-256:]))
    return h


def _get_nc(shared, repeat=1):
    key = (N_LAYERS, B_LOCAL, S, repeat, _weights_key(shared))
    if key not in _NC_CACHE:
        _NC_CACHE[key] = build_nc(
            N_LAYERS, B_LOCAL, S, shared, num_devices=N_CORES, repeat=repeat
        )
    return _NC_CACHE[key]


def make_in_maps(**inputs):
    h = np.asarray(inputs["hidden_states"], np.float32)
    return [
        {"hT": _prep_hT(h[c * B_LOCAL : (c + 1) * B_LOCAL])} for c in range(N_CORES)
    ]


def gather_output(results):
    return np.concatenate(
        [_unprep_outT(results[c]["outT"], B_LOCAL, S) for c in range(N_CORES)],
        axis=0,
    )


def kernel(**inputs):
    from concourse.bass_utils import run_bass_kernel_spmd

    shared = _prep_shared(inputs)
    nc = _get_nc(shared)
    in_maps = make_in_maps(**inputs)
    res = run_bass_kernel_spmd(nc, in_maps, list(range(N_CORES)))
    return gather_output(res.results)


# revision 18
# speedup vs baseline: 17.5036x; 1.0531x over previous
"""Trainium2 Bass kernel for a 6-layer BERT encoder (nn_BertEncoder).

Strategy: data-parallel over batch across 8 NeuronCores (16 batches -> 2 per
core), no collectives.  Weights are embedded in the NEFF as Const tensors
(inline_tensor) so each call only ships the hidden states.  Inside each core
the encoder runs with feature-major ("transposed") activations so every
projection uses the weight as the PE stationary operand with zero transposes.

Attention: transposed scores per head; per head-pair the even head's softmax
denominator comes free from a ones-column appended to V, the odd head's from
M=1 ones matmuls into partition 96 of the even context bank.  tile_position
keyed matmuls broadcast the reciprocal rows straight into the right partition
halves, so the whole softmax tail is DMA-free and partition-aligned.

LayerNorm: per 512-token block, pipelined with the producing matmul phase;
the per-token scale/shift rows are broadcast to [128,512] PSUM tiles via
K=1 matmuls (a = rstd, b = -mean*rstd) and applied with two tensor-tensor
ops per slab, gain/bias folded into a per-partition tensor-scalar.
"""

import sys

sys.path.insert(0, "/opt/trn_rl_repo")

from contextlib import ExitStack

import numpy as np
import ml_dtypes

import concourse.bass as bass
import concourse.mybir as mybir
import concourse.tile as tile
from concourse.vector_clock import ScopedClock, VectorClock

F32 = mybir.dt.float32
BF16 = mybir.dt.bfloat16
AF = mybir.ActivationFunctionType
ALU = mybir.AluOpType

H = 768
I = 3072
NH = 12
DH = 64
KH = H // 128  # 6 feature slabs
KI = I // 128  # 24 intermediate chunks
NP = NH // 2  # 6 head pairs
PW = 2 * DH + 1  # per-pair v_aug width (even v + ones col + odd v)
EPS = 1e-12


class SplitDrainTileContext(tile.TileContext):
    """TileContext whose kernel-tail drain splits its semaphore waits across
    several SP nops -- this walrus build rejects >1 sync wait on a TPB_CTRL
    (Drain/NoOp) instruction."""

    def _drain_and_barrier(self, tick_clock, wait_clock):
        gc = tick_clock.global_clock
        nprocs = len(gc)
        procs = [p for p in range(nprocs) if gc[p] > 0]
        covered = [0] * nprocs
        for p in procs:
            req = [0] * nprocs
            req[p] = gc[p]
            nop_inst = self.nc.sync.nop(nofuse=True)
            wait_clock.add_sem_waits(
                nop_inst.ins,
                ScopedClock({None: VectorClock(req)}),
                ScopedClock({None: VectorClock(list(covered))}),
            )
            covered[p] = gc[p]
        drain_inst = self.nc.sync.drain()
        wait_clock.add_sem_waits(
            drain_inst.ins,
            ScopedClock({None: gc.copy()}),
            ScopedClock({None: VectorClock(list(covered))}),
        )
        self.nc.all_engine_barrier()
        assert self.sems is not None
        popped = self.nc._tile_sem_poison_stack.pop()
        assert popped is self._sem_poison
        self.nc.clear_and_free_semaphores(list(self.sems.allocated().values()))
        self.nc.all_engine_barrier()


def split_multi_waits(nc):
    """Walrus in this container accepts at most ONE sync wait per
    instruction.  Split every instruction carrying N>1 waits into N-1
    same-engine NOPs (each holding one wait) followed by the original
    instruction with the final wait."""
    f = nc.m.functions[0]
    n_split = 0
    for bb in f.blocks:
        insts = list(bb.instructions)
        out = []
        changed = False
        for inst in insts:
            si = inst.sync_info
            if si is not None and len(si.on_wait) > 1:
                waits = list(si.on_wait)
                for j, w in enumerate(waits[:-1]):
                    nop = mybir.InstNoOp(
                        name=f"{inst.name}_sw{j}",
                        engine=inst.engine,
                        sync_info=mybir.SyncInfo(on_wait=[w], on_update=[]),
                        bass_nofuse=True,
                    )
                    out.append(nop)
                inst.sync_info = mybir.SyncInfo(
                    on_wait=[waits[-1]], on_update=list(si.on_update)
                )
                n_split += 1
                changed = True
            out.append(inst)
        if changed:
            bb.instructions = out
    for bb in nc.m.functions[0].blocks:
        for inst in bb.instructions:
            si = inst.sync_info
            assert si is None or len(si.on_wait) <= 1, (
                f"multi-wait survived on {inst.name}"
            )
    return n_split


_PHASE_MARKS = []  # (phase_label, first_inst_id) — analysis only


def _mark(nc, label):
    _PHASE_MARKS.append((label, nc.next_id()))


def build_nc(
    n_layers: int,
    b_local: int,
    s: int,
    weights: dict,
    num_devices: int = 8,
    split_waits: bool = True,
    repeat: int = 1,
):
    """Build the per-core Bass module.  `weights` holds the prepped host
    arrays (see _prep_shared); they are embedded in the NEFF as Const
    tensors.  Only hT (transposed hidden states) is a per-call input."""
    T = b_local * s
    nc = bass.Bass(
        "TRN2", target_bir_lowering=False, debug=False, num_devices=num_devices
    )

    hT = nc.dram_tensor("hT", [128, KH, T], F32, kind="ExternalInput").ap()
    w = {k: nc.inline_tensor(v, name=k).ap() for k, v in weights.items()}
    outT = nc.dram_tensor("outT", [128, KH, T], F32, kind="ExternalOutput").ap()

    with SplitDrainTileContext(nc) as tc:
        carry = []
        for r in range(repeat):
            carry = build_body(
                tc, n_layers, b_local, s, hT, w, outT, carry, last_pass=(r == repeat - 1)
            )
    if split_waits:
        split_multi_waits(nc)
    return nc


def build_body(tc, n_layers, b_local, s, hT, w, outT, carry_pools=(), last_pass=True):
    nc = tc.nc
    T = b_local * s
    NT = T // 128
    NS = T // 512  # = b_local (one 512-token block per batch)
    SC = s // 128  # key chunks per batch = 4
    wq, wk, wv, wo, wi, wo2 = w["wq"], w["wk"], w["wv"], w["wo"], w["wi"], w["wo2"]
    cols, bvrow = w["cols"], w["bvrow"]

    ctx_stack = ExitStack()
    ec = ctx_stack.enter_context
    big = ec(tc.tile_pool(name="big_f32", bufs=2))  # h / x1 / x2 (f32)
    actb = ec(tc.tile_pool(name="act_bf16", bufs=2))  # ctx
    qkp = ec(tc.tile_pool(name="qk", bufs=2))
    vap = ec(tc.tile_pool(name="vaug", bufs=1))
    expp = ec(tc.tile_pool(name="exp", bufs=10))
    rbp = ec(tc.tile_pool(name="rbcast", bufs=2))
    lnw = ec(tc.tile_pool(name="lnwork", bufs=1))
    rows = ec(tc.tile_pool(name="rows", bufs=2))
    rrp = ec(tc.tile_pool(name="rrp", bufs=2))
    wpj = ec(tc.tile_pool(name="wproj", bufs=3))
    wib = ec(tc.tile_pool(name="wiblk", bufs=2))
    w2p = ec(tc.tile_pool(name="wo2", bufs=2))
    bcol = ec(tc.tile_pool(name="bcols", bufs=2))
    gel = ec(tc.tile_pool(name="gelu", bufs=4))
    ones_pool = ec(tc.tile_pool(name="ones", bufs=1))

    ones_bf16 = ones_pool.tile([128, 1], BF16)
    nc.vector.memset(ones_bf16, 1.0)
    eps_t = ones_pool.tile([128, 1], F32)
    nc.vector.memset(eps_t, EPS)
    ones_f32row = ones_pool.tile([1, 128], F32)
    nc.vector.memset(ones_f32row, 1.0)
    neg_f32row = ones_pool.tile([1, 128], F32)
    nc.vector.memset(neg_f32row, -1.0)
    # bf16 ones block: rows 64 and 96 feed the K=1 reciprocal broadcasts
    ones_blk = ones_pool.tile([128, DH], BF16)
    nc.vector.memset(ones_blk, 1.0)

    # ---- initial hidden state (bf16 residual stream; SWDGE casts) -------
    h = big.tile([128, KH, T], BF16, tag="bigbuf")
    nc.gpsimd.dma_start(out=h[:], in_=hT)

    def ln_block(x, n, g_col, b_col, psrows, psbc, last=False):
        """LayerNorm of token block n (512 cols) of x [128,KH,T] bf16 in
        place (stats read the bf16 stream directly)."""
        tsl = bass.ts(n, 512)
        xsq = lnw.tile([128, KH, 512], BF16, tag="lnxsq")
        nc.vector.tensor_tensor(xsq[:], x[:, :, tsl], x[:, :, tsl], op=ALU.mult)
        ps_sum = psrows.tile([1, 512], F32, tag="lnrow")
        for k in range(KH):
            nc.tensor.matmul(
                ps_sum, lhsT=ones_bf16, rhs=x[:, k, tsl],
                start=(k == 0), stop=(k == KH - 1),
            )
        ps_sq = psrows.tile([1, 512], F32, tag="lnrow")
        for k in range(KH):
            nc.tensor.matmul(
                ps_sq, lhsT=ones_bf16, rhs=xsq[:, k, :],
                start=(k == 0), stop=(k == KH - 1),
            )
        srow = rows.tile([1, 3 * 512], F32, tag="srow", bufs=1)
        mean_r = srow[:, 0:512]
        msq_r = srow[:, 512:1024]
        var_r = srow[:, 1024:1536]
        mr_r = mean_r  # mean*rstd overwrites the mean slot in place
        nc.scalar.activation(out=mean_r, in_=ps_sum, func=AF.Copy, scale=1.0 / H)
        nc.scalar.activation(out=msq_r, in_=ps_sq, func=AF.Copy, scale=1.0 / H)
        nc.vector.tensor_tensor(var_r, mean_r, mean_r, op=ALU.mult)
        nc.vector.tensor_tensor(var_r, msq_r, var_r, op=ALU.subtract)
        # rstd = exp(-0.5*ln(var+eps)): stays in the natural_log_exp ACT
        # table set (shared with attention's Exp) -- avoids the ~2.7us
        # sqrt table-set switch twice per layer
        rstd_r = msq_r
        nc.scalar.activation(out=var_r, in_=var_r, func=AF.Ln, bias=eps_t[0:1, :])
        nc.scalar.activation(out=rstd_r, in_=var_r, func=AF.Exp, scale=-0.5)
        nc.vector.tensor_tensor(mr_r, mean_r, rstd_r, op=ALU.mult)
        # broadcast a = rstd, b = -mean*rstd to [128,512] PSUM tiles
        ps_a = psbc.tile([128, 512], F32, tag="lnbc")
        nc.tensor.matmul(ps_a, lhsT=ones_f32row, rhs=rstd_r, start=True, stop=True)
        ps_b = psbc.tile([128, 512], F32, tag="lnbc")
        nc.tensor.matmul(ps_b, lhsT=neg_f32row, rhs=mr_r, start=True, stop=True)
        for k in range(KH):
            xs = x[:, k, tsl]
            nc.vector.tensor_tensor(xs, xs, ps_a, op=ALU.mult)
            nc.vector.tensor_tensor(xs, xs, ps_b, op=ALU.add)
            nc.vector.tensor_scalar(
                out=xs,
                in0=xs,
                scalar1=g_col[:, k : k + 1],
                scalar2=b_col[:, k : k + 1],
                op0=ALU.mult,
                op1=ALU.add,
            )

    pend_ln_pools = list(carry_pools)  # prev LN2 pools, closed after next QKV

    for l in range(n_layers):
        _mark(nc, f"L{l}_const")
        cols_t = bcol.tile([128, 8 * KH + KI], F32, tag="cols")
        nc.sync.dma_start(out=cols_t[:], in_=cols[l])
        cq_t = cols_t[:, 0 * KH : 1 * KH]
        ck_t = cols_t[:, 1 * KH : 2 * KH]
        co_t = cols_t[:, 2 * KH : 3 * KH]
        cg1_t = cols_t[:, 3 * KH : 4 * KH]
        cb1_t = cols_t[:, 4 * KH : 5 * KH]
        co2_t = cols_t[:, 5 * KH : 6 * KH]
        cg2_t = cols_t[:, 6 * KH : 7 * KH]
        cb2_t = cols_t[:, 7 * KH : 8 * KH]
        ci_t = cols_t[:, 8 * KH : 8 * KH + KI]
        bvbuf = rows.tile([1, H], F32, tag="bvrow", bufs=1)
        bv_r = bvbuf[0:1, 0:H]
        nc.sync.dma_start(out=bv_r, in_=bvrow[l])
        bv_b = bcol.tile([128, H], F32, tag="bvb", bufs=1)

        wq_t = wpj.tile([128, KH, H], BF16, tag="wpj", name=f"wq_{l}")
        nc.sync.dma_start(out=wq_t[:], in_=wq[l].rearrange("k p h -> p k h"))
        wk_t = wpj.tile([128, KH, H], BF16, tag="wpj", name=f"wk_{l}")
        nc.sync.dma_start(out=wk_t[:], in_=wk[l].rearrange("k p h -> p k h"))
        wv_t = wpj.tile([128, KH, H], BF16, tag="wpj", name=f"wv_{l}")
        nc.sync.dma_start(out=wv_t[:], in_=wv[l].rearrange("k p h -> p k h"))
        wo_t = wpj.tile([128, KH, H], BF16, tag="wpj", name=f"wo_{l}")
        nc.sync.dma_start(out=wo_t[:], in_=wo[l].rearrange("k p h -> p k h"))
        wq_s = [wq_t[:, k, :] for k in range(KH)]
        wk_s = [wk_t[:, k, :] for k in range(KH)]
        wv_s = [wv_t[:, k, :] for k in range(KH)]
        wo_s = [wo_t[:, k, :] for k in range(KH)]

        # ---- phase A: QKV projections -----------------------------------
        _mark(nc, f"L{l}_A_qkv")
        qT = qkp.tile([128, KH, T], BF16, tag="qk")
        kT = qkp.tile([128, KH, T], BF16, tag="qk")
        v_aug = vap.tile([128, NT, NP * PW], BF16, tag="vaug")
        psA_cm = tc.tile_pool(name=f"psA_{l}", bufs=4, space="PSUM")
        psA = psA_cm.__enter__()
        for f0, fw in ((0, 512), (512, 256)):
            ps_bv = psA.tile([128, 512], F32, tag="psA")
            nc.tensor.matmul(
                ps_bv[:, 0:fw],
                lhsT=ones_f32row,
                rhs=bv_r[:, f0 : f0 + fw],
                start=True,
                stop=True,
            )
            nc.vector.tensor_copy(out=bv_b[:, f0 : f0 + fw], in_=ps_bv[:, 0:fw])
        for n in range(NS):
            tsl = bass.ts(n, 512)
            for m in range(KH):
                msl = bass.ts(m, 128)
                ps = psA.tile([128, 512], F32, tag="psA")
                for k in range(KH):
                    nc.tensor.matmul(
                        ps,
                        lhsT=wq_s[k][:, msl],
                        rhs=h[:, k, tsl],
                        start=(k == 0),
                        stop=(k == KH - 1),
                    )
                nc.scalar.activation(
                    out=qT[:, m, tsl], in_=ps, func=AF.Identity,
                    bias=cq_t[:, m : m + 1],
                )
                ps2 = psA.tile([128, 512], F32, tag="psA")
                for k in range(KH):
                    nc.tensor.matmul(
                        ps2,
                        lhsT=wk_s[k][:, msl],
                        rhs=h[:, k, tsl],
                        start=(k == 0),
                        stop=(k == KH - 1),
                    )
                nc.scalar.activation(
                    out=kT[:, m, tsl], in_=ps2, func=AF.Identity,
                    bias=ck_t[:, m : m + 1],
                )
        # V in natural layout (tokens on partitions); per pair:
        # [even v (64) | ones col | odd v (64)]
        for mt in range(NT):
            tsl = bass.ts(mt, 128)
            vv = v_aug[:, mt, :].rearrange("p (pr w) -> p pr w", w=PW)
            nc.vector.memset(vv[:, :, DH : DH + 1], 1.0)
            for f0, fw in ((0, 512), (512, 256)):
                npr = fw // 128  # pairs in this feature range
                pr0 = f0 // 128
                ps = psA.tile([128, 512], F32, tag="psA")
                for k in range(KH):
                    nc.tensor.matmul(
                        ps[:, 0:fw],
                        lhsT=h[:, k, tsl],
                        rhs=wv_s[k][:, f0 : f0 + fw],
                        start=(k == 0),
                        stop=(k == KH - 1),
                    )
                psv = ps[:, 0:fw].rearrange("p (pr two d) -> p pr two d", two=2, d=DH)
                bvv = bv_b[:, f0 : f0 + fw].rearrange(
                    "p (pr two d) -> p pr two d", two=2, d=DH
                )
                dst = v_aug[:, mt, pr0 * PW : (pr0 + npr) * PW].rearrange(
                    "p (pr w) -> p pr w", w=PW
                )
                nc.vector.tensor_tensor(
                    dst[:, :, 0:DH], psv[:, :, 0, :], bvv[:, :, 0, :], op=ALU.add
                )
                nc.vector.tensor_tensor(
                    dst[:, :, DH + 1 : PW], psv[:, :, 1, :], bvv[:, :, 1, :],
                    op=ALU.add,
                )
        psA_cm.__exit__(None, None, None)
        # close previous layer's LN2 psum pools (LIFO: they were entered
        # before psA) — QKV above overlaps the tail of the previous LN2
        for cm in reversed(pend_ln_pools):
            cm.__exit__(None, None, None)
        pend_ln_pools = []

        # ---- phase B: attention -----------------------------------------
        _mark(nc, f"L{l}_B_attn")
        ctx = actb.tile([128, KH, T], BF16, tag="actb")
        psO_cm = tc.tile_pool(name=f"psO_{l}", bufs=2, space="PSUM")
        psO = psO_cm.__enter__()
        psS_cm = tc.tile_pool(name=f"psS_{l}", bufs=3, space="PSUM")
        psS = psS_cm.__enter__()
        psC_cm = tc.tile_pool(name=f"psC_{l}", bufs=3, space="PSUM")
        psC = psC_cm.__enter__()
        for b in range(b_local):
            ssl = bass.ds(b * s, s)
            for pr in range(NP):
                sl = pr
                qh_e = qT[0:DH, sl, ssl]
                qh_o = qT[DH:128, sl, ssl]
                ets = {0: [], 1: []}  # parity -> [et [128,512] bf16]
                # interleave parities: adjacent matmuls hit disjoint PE row
                # groups (rows 0-63 vs 64-127) and overlap in the array
                for tci in range(SC):
                    for par, qh in ((0, qh_e), (1, qh_o)):
                        kv = kT[par * DH : par * DH + DH, sl, :]
                        ps = psS.tile([128, 512], F32, tag="psS")
                        nc.tensor.matmul(
                            ps,
                            lhsT=kv[:, bass.ds(b * s + tci * 128, 128)],
                            rhs=qh,
                            start=True,
                            stop=True,
                        )
                        et = expp.tile([128, 512], BF16, tag="exp")
                        nc.scalar.activation(
                            out=et, in_=ps, func=AF.Exp, scale=1.0 / 8.0
                        )
                        ets[par].append(et)
                p0 = pr * PW
                pc_e = psC.tile([128, 512], F32, tag="psC")
                for tci in range(SC):
                    mt = b * SC + tci
                    nc.tensor.matmul(
                        pc_e[0 : DH + 1],
                        lhsT=v_aug[:, mt, p0 : p0 + DH + 1],
                        rhs=ets[0][tci],
                        start=(tci == 0),
                        stop=(tci == SC - 1),
                    )
                pc_o = psC.tile([128, 512], F32, tag="psC")
                for tci in range(SC):
                    mt = b * SC + tci
                    nc.tensor.matmul(
                        pc_o[DH:128],
                        lhsT=v_aug[:, mt, p0 + DH + 1 : p0 + PW],
                        rhs=ets[1][tci],
                        start=(tci == 0),
                        stop=(tci == SC - 1),
                    )
                # odd-head denominator into partition 96 of the even bank
                for tci in range(SC):
                    nc.tensor.matmul(
                        pc_e[96:97],
                        lhsT=ones_bf16,
                        rhs=ets[1][tci],
                        start=(tci == 0),
                        stop=(tci == SC - 1),
                        tile_position=(0, 96),
                        skip_group_check=True,
                    )
                # reciprocal rows (partitions 64 and 96), bf16
                rrb = rrp.tile([128, 512], BF16, tag="rrb")
                with nc.allow_low_precision(reason="softmax denom in bf16"):
                    nc.vector.reciprocal(rrb[DH : DH + 1, :], pc_e[DH : DH + 1, :])
                    nc.vector.reciprocal(rrb[96:97, :], pc_e[96:97, :])
                # broadcast 1/denom to the head's partition half
                pcb = psC.tile([128, 512], F32, tag="psC")
                nc.tensor.matmul(
                    pcb[0:DH],
                    lhsT=ones_blk[DH : DH + 1, :],
                    rhs=rrb[DH : DH + 1, :],
                    start=True,
                    stop=True,
                )
                nc.tensor.matmul(
                    pcb[DH:128],
                    lhsT=ones_blk[96:97, :],
                    rhs=rrb[96:97, :],
                    start=True,
                    stop=True,
                    tile_position=(96, DH),
                    skip_group_check=True,
                )
                rb = rbp.tile([128, 512], F32, tag="rb")
                nc.vector.tensor_copy(out=rb[0:DH], in_=pcb[0:DH])
                nc.vector.tensor_copy(out=rb[DH:128], in_=pcb[DH:128])
                nc.vector.tensor_tensor(
                    ctx[0:DH, sl, ssl], pc_e[0:DH], rb[0:DH], op=ALU.mult
                )
                nc.vector.tensor_tensor(
                    ctx[DH:128, sl, ssl], pc_o[DH:128], rb[DH:128], op=ALU.mult
                )
        psC_cm.__exit__(None, None, None)
        psS_cm.__exit__(None, None, None)
        # psO stays open: oproj(0) overlaps attn(1); exits after LN1

        # ---- phase C: output proj + residual + LN1 (per block) ----------
        _mark(nc, f"L{l}_C_oproj")
        x1 = big.tile([128, KH, T], BF16, tag="bigbuf")
        psL1r_cm = tc.tile_pool(name=f"psL1r_{l}", bufs=2, space="PSUM")
        psL1r = psL1r_cm.__enter__()
        psL1b_cm = tc.tile_pool(name=f"psL1b_{l}", bufs=2, space="PSUM")
        psL1b = psL1b_cm.__enter__()
        for n in range(NS):
            tsl = bass.ts(n, 512)
            for m in range(KH):
                msl = bass.ts(m, 128)
                ps = psO.tile([128, 512], F32, tag="psO")
                for k in range(KH):
                    nc.tensor.matmul(
                        ps,
                        lhsT=wo_s[k][:, msl],
                        rhs=ctx[:, k, tsl],
                        start=(k == 0),
                        stop=(k == KH - 1),
                    )
                nc.scalar.activation(
                    out=x1[:, m, tsl], in_=ps, func=AF.Identity,
                    bias=co_t[:, m : m + 1],
                )
                nc.vector.tensor_tensor(
                    x1[:, m, tsl], x1[:, m, tsl], h[:, m, tsl], op=ALU.add
                )
            ln_block(x1, n, cg1_t, cb1_t, psL1r, psL1b)
        psL1b_cm.__exit__(None, None, None)
        psL1r_cm.__exit__(None, None, None)
        psO_cm.__exit__(None, None, None)
        psG_cm = tc.tile_pool(name=f"psG_{l}", bufs=2, space="PSUM")
        psG = psG_cm.__enter__()

        # ---- phase D: FFN (fused over I-chunks) + residual --------------
        _mark(nc, f"L{l}_D_ffn")
        x2 = big.tile([128, KH, T], BF16, tag="bigbuf")
        psF_cm = tc.tile_pool(name=f"psF_{l}", bufs=6, space="PSUM")
        psF = psF_cm.__enter__()
        for n in range(NS):
            tsl = bass.ts(n, 512)
            outps = [
                psF.tile([128, 512], F32, tag="psF", name=f"outps{n}_{m}")
                for m in range(KH)
            ]
            IG = 2  # i-chunks per weight-block DMA
            pend = None
            for ig in range(KI // IG):
                wi_blk = wib.tile(
                    [128, KH, IG * 128], BF16, tag="wib", name=f"wib{n}_{ig}"
                )
                nc.sync.dma_start(
                    out=wi_blk[:],
                    in_=wi[l, :, :, bass.ts(ig, IG * 128)].rearrange(
                        "k p w -> p k w"
                    ),
                )
                w2_blk = w2p.tile([128, IG, H], BF16, tag="w2", name=f"w2b{n}_{ig}")
                nc.sync.dma_start(
                    out=w2_blk[:],
                    in_=wo2[l, ig * IG : (ig + 1) * IG].rearrange("i p h -> p i h"),
                )
                for ii in range(IG):
                    i = ig * IG + ii
                    psg = psG.tile([128, 512], F32, tag="psG")
                    for k in range(KH):
                        nc.tensor.matmul(
                            psg,
                            lhsT=wi_blk[:, k, bass.ts(ii, 128)],
                            rhs=x1[:, k, tsl],
                            start=(k == 0),
                            stop=(k == KH - 1),
                        )
                    gl = gel.tile([128, 512], BF16, tag="gel")
                    nc.scalar.activation(
                        out=gl, in_=psg, func=AF.Gelu, bias=ci_t[:, i : i + 1]
                    )
                    if pend is not None:
                        pgl, pw2, pii, pi = pend
                        for m in range(KH):
                            nc.tensor.matmul(
                                outps[m],
                                lhsT=pw2[:, pii, bass.ts(m, 128)],
                                rhs=pgl,
                                start=(pi == 0),
                                stop=False,
                            )
                    pend = (gl, w2_blk, ii, i)
            pgl, pw2, pii, pi = pend
            for m in range(KH):
                nc.tensor.matmul(
                    outps[m],
                    lhsT=pw2[:, pii, bass.ts(m, 128)],
                    rhs=pgl,
                    start=False,
                    stop=True,
                )
            for m in range(KH):
                nc.scalar.activation(
                    out=x2[:, m, tsl], in_=outps[m], func=AF.Identity,
                    bias=co2_t[:, m : m + 1],
                )
                nc.vector.tensor_tensor(
                    x2[:, m, tsl], x2[:, m, tsl], x1[:, m, tsl], op=ALU.add
                )

        # ---- phase E: LN2 (per block, overlaps next layer's QKV) --------
        _mark(nc, f"L{l}_E_ln2")
        last = l == n_layers - 1
        psF_cm.__exit__(None, None, None)
        psG_cm.__exit__(None, None, None)
        psL2r_cm = tc.tile_pool(name=f"psL2r_{l}", bufs=2, space="PSUM")
        psL2r = psL2r_cm.__enter__()
        psL2b_cm = tc.tile_pool(name=f"psL2b_{l}", bufs=2, space="PSUM")
        psL2b = psL2b_cm.__enter__()
        for n in range(NS):
            ln_block(x2, n, cg2_t, cb2_t, psL2r, psL2b, last=last)
        pend_ln_pools = [psL2r_cm, psL2b_cm]
        h = x2

    nc.gpsimd.dma_start(out=outT, in_=h[:])
    if last_pass:
        for cm in reversed(pend_ln_pools):
            cm.__exit__(None, None, None)
        pend_ln_pools = []
    ctx_stack.close()
    return pend_ln_pools


# ---------------------------------------------------------------------------
# host side
# ---------------------------------------------------------------------------

BF = ml_dtypes.bfloat16
N_CORES = 8
B, S = 16, 512
B_LOCAL = B // N_CORES
N_LAYERS = 6


def _slabs(w, kdim):  # [L, in, out] -> [L, kdim, 128, out] bf16
    return np.ascontiguousarray(
        np.asarray(w, np.float32).reshape(N_LAYERS, kdim, 128, -1)
    ).astype(BF)


def _cols(b, kdim):  # [L, kdim*128] -> [L, 128, kdim] f32
    return np.ascontiguousarray(
        np.asarray(b, np.float32).reshape(N_LAYERS, kdim, 128).transpose(0, 2, 1)
    )


def _prep_shared(inputs):
    packed = np.concatenate(
        [
            _cols(inputs["bq"], KH),
            _cols(inputs["bk"], KH),
            _cols(inputs["bo"], KH),
            _cols(inputs["g1"], KH),
            _cols(inputs["b1"], KH),
            _cols(inputs["bo2"], KH),
            _cols(inputs["g2"], KH),
            _cols(inputs["b2"], KH),
            _cols(inputs["bi"], KI),
        ],
        axis=2,
    )
    return {
        "wq": _slabs(inputs["Wq"], KH),
        "wk": _slabs(inputs["Wk"], KH),
        "wv": _slabs(inputs["Wv"], KH),
        "wo": _slabs(inputs["Wo"], KH),
        "wi": _slabs(inputs["Wi"], KH),
        "wo2": _slabs(inputs["Wo2"], KI),
        "cols": np.ascontiguousarray(packed),
        "bvrow": np.ascontiguousarray(
            np.asarray(inputs["bv"], np.float32).reshape(N_LAYERS, 1, H)
        ),
    }


def _prep_hT(h_shard):
    b, s, _ = h_shard.shape
    T = b * s
    return np.ascontiguousarray(
        np.asarray(h_shard, np.float32)
        .reshape(T, H)
        .T.reshape(KH, 128, T)
        .transpose(1, 0, 2)
    )


def _unprep_outT(outT, b, s):
    T = b * s
    return np.ascontiguousarray(
        np.asarray(outT).transpose(1, 0, 2).reshape(H, T).T.reshape(b, s, H)
    )


_NC_CACHE = {}


def _weights_key(shared):
    h = 0
    for k in sorted(shared):
        a = shared[k]
        h ^= hash((k, a.shape, a.dtype.str, a.tobytes()[:256], a.tobytes()[-256:]))
    return h


def _get_nc(shared, repeat=1):
    key = (N_LAYERS, B_LOCAL, S, repeat, _weights_key(shared))
    if key not in _NC_CACHE:
        _NC_CACHE[key] = build_nc(
            N_LAYERS, B_LOCAL, S, shared, num_devices=N_CORES, repeat=repeat
        )
    return _NC_CACHE[key]


def make_in_maps(**inputs):
    h = np.asarray(inputs["hidden_states"], np.float32)
    return [
        {"hT": _prep_hT(h[c * B_LOCAL : (c + 1) * B_LOCAL])} for c in range(N_CORES)
    ]


def gather_output(results):
    return np.concatenate(
        [_unprep_outT(results[c]["outT"], B_LOCAL, S) for c in range(N_CORES)],
        axis=0,
    )


def kernel(**inputs):
    from concourse.bass_utils import run_bass_kernel_spmd

    shared = _prep_shared(inputs)
    nc = _get_nc(shared)
    in_maps = make_in_maps(**inputs)
    res = run_bass_kernel_spmd(nc, in_maps, list(range(N_CORES)))
    return gather_output(res.results)
